# revision 25
# baseline (speedup 1.0000x reference)
"""Multi-head causal attention (B=2, S=2048, D=1024, H=16) on 8 Trainium2
NeuronCores.

Sharding: data-parallel over the 2 batches x tensor-parallel over 4 head
groups (4 heads each).  Core c handles batch c//4, heads [4*(c%4), 4*(c%4)+4).
Each core computes its Q/K/V projections from column shards of Wq/Wk/Wv,
runs causal attention for its heads, and applies its row shard of Wo,
producing a partial (D, S) output in bf16.  The host sums the 4 partials
per batch in f32 and adds the output bias.

On-core layout: activations are kept transposed (feature dim on SBUF
partitions, sequence on the free axis) so every matmul's operands are
already in the (K x M)/(K x N) form the PE array wants, and the softmax
denominator comes free from an extra ones-row appended to V.

Schedule: phases are interleaved per 512-query block
(proj(0), attn(0), proj(1), outproj(0), attn(1), ...) and the attention
inner loop issues score matmuls one group ahead of the PV matmuls, so the
tensor engine never waits on the scalar-engine exp and stays continuously
busy (keeps the PE at its top p-state clock).  The causal mask is applied
by multiplying the exp'd scores with precomputed bf16 0/1 tiles on the
vector engine; softmax division uses the fast approximate reciprocal.
"""

import sys

sys.path.insert(0, "/opt/trn_rl_repo")

import numpy as np

B, S, D, H = 2, 2048, 1024, 16
DK = D // H            # 64 head dim
NCORES = 8
NGROUPS = 4            # head groups (tensor parallel)
NH = H // NGROUPS      # 4 heads per core
DHL = NH * DK          # 256 local head dims per core
P = 128
DC = D // P            # 8 contraction chunks over D
HC = DHL // P          # 2 local head-dim chunks
SB = 512               # query block (matmul moving free size)
NSB = S // SB          # 4
SCK = S // P           # 16 key chunks
HPC = P // DK          # 2 heads per head-dim chunk
G = 2                  # score chunks per exp group (2 PSUM banks)

# fp8(e4m3) DoubleRow matmuls for the Q/K projections (V and the output
# projection stay bf16 — fp8 there fails the 2e-2 accuracy gate).
QK_FP8 = True

_CACHE = {}


def _build_nc(causal, qk_fp8):
    import concourse.bass as bass
    import concourse.bacc as bacc
    import concourse.mybir as mybir
    import concourse.tile as tile
    from contextlib import ExitStack

    f32 = mybir.dt.float32
    mmdt = mybir.dt.bfloat16
    qkdt = mybir.dt.float8e4 if qk_fp8 else mmdt
    DR = mybir.MatmulPerfMode.DoubleRow if qk_fp8 else None
    Exp = mybir.ActivationFunctionType.Exp
    is_ge = mybir.AluOpType.is_ge

    nc = bacc.Bacc(None, target_bir_lowering=False, debug=False)

    # x pre-arranged on host to [P, NSB, DC, SB]: per-partition contiguous
    # 8KB blocks per sequence block -> clean 2D DMA.
    xq_d = nc.dram_tensor("xq_a", [P, NSB * DC * SB], qkdt, kind="ExternalInput")
    xk_d = nc.dram_tensor("xk_a", [P, NSB * DC * SB], qkdt, kind="ExternalInput")
    xv_d = nc.dram_tensor("xv_a", [P, NSB * DC * SB], mmdt, kind="ExternalInput")
    # weights pre-arranged on host to the exact SBUF layouts
    wq_d = nc.dram_tensor("wq_a", [P, DC * DHL], qkdt, kind="ExternalInput")
    wk_d = nc.dram_tensor("wk_a", [P, DC * DHL], qkdt, kind="ExternalInput")
    wv_d = nc.dram_tensor("wv_a", [P, DC * DHL], mmdt, kind="ExternalInput")
    wo_d = nc.dram_tensor("wo_a", [P, HC * D], mmdt, kind="ExternalInput")
    bq_d = nc.dram_tensor("bq_a", [P, HC], f32, kind="ExternalInput")
    bk_d = nc.dram_tensor("bk_a", [P, HC], f32, kind="ExternalInput")
    bv_d = nc.dram_tensor("bv_a", [1, DHL], f32, kind="ExternalInput")
    # bf16 partial output, same [P, NSB, DC, SB] arrangement
    out_d = nc.dram_tensor("out_a", [P, NSB * DC * SB], mmdt,
                           kind="ExternalOutput")

    xq_r = xq_d[:].rearrange("p (b c s) -> p b c s", b=NSB, c=DC)
    xk_r = xk_d[:].rearrange("p (b c s) -> p b c s", b=NSB, c=DC)
    xv_r = xv_d[:].rearrange("p (b c s) -> p b c s", b=NSB, c=DC)
    out_r = out_d[:].rearrange("p (b c s) -> p b c s", b=NSB, c=DC)

    inv_sqrt_dk = 1.0 / float(np.sqrt(DK))

    with tile.TileContext(nc) as tc, ExitStack() as ctx:
        consts = ctx.enter_context(tc.tile_pool(name="consts", bufs=1))
        xpool = ctx.enter_context(tc.tile_pool(name="xpool", bufs=6))
        exp_pool = ctx.enter_context(tc.tile_pool(name="exp_pool", bufs=3))
        small = ctx.enter_context(tc.tile_pool(name="small", bufs=6))
        opool = ctx.enter_context(tc.tile_pool(name="opool", bufs=2))
        proj_ps = ctx.enter_context(
            tc.tile_pool(name="proj_ps", bufs=2, space="PSUM"))
        sc_ps_pool = ctx.enter_context(
            tc.tile_pool(name="sc_ps", bufs=2, space="PSUM"))
        pv_ps_pool = ctx.enter_context(
            tc.tile_pool(name="pv_ps", bufs=2, space="PSUM"))

        # --- resident tensors ---
        wq_sb = consts.tile([P, DC, DHL], qkdt)
        wk_sb = consts.tile([P, DC, DHL], qkdt)
        wv_sb = consts.tile([P, DC, DHL], mmdt)
        wo_sb = consts.tile([P, HC, D], mmdt)
        bq_sb = consts.tile([P, HC], f32)
        bk_sb = consts.tile([P, HC], f32)
        bv_row = consts.tile([1, DHL], f32)
        bv_bc = consts.tile([P, DHL], f32)
        q_sb = consts.tile([P, HC, S], mmdt)
        k_sb = consts.tile([P, HC, S], mmdt)
        v_aug = consts.tile([P, SCK, NH, DK + 1], mmdt)
        attn_sb = consts.tile([P, HC, S], mmdt)
        # causal 0/1 masks for the two diagonal groups: plane j2 of mask
        # tile m keeps (p, i) when i - p - 128*(2*m + j2) >= 0
        masks = [consts.tile([P, G, SB], mmdt, name=f"mask{m}")
                 for m in range(2)]

        def load_x(sbi, split=False):
            tiles = {}
            for name, src, dt_ in (("q", xq_r, qkdt), ("k", xk_r, qkdt),
                                   ("v", xv_r, mmdt)):
                t = xpool.tile([P, DC, SB], dt_, name=f"x{name}{sbi}",
                               tag="xs")
                # at startup, route xv over the scalar HWDGE queue so it
                # overlaps the q/k loads on the sync queue
                eng = nc.scalar if (split and name == "v") else nc.sync
                eng.dma_start(t[:], src[:, sbi])
                tiles[name] = t
            return tiles

        # --- startup: weight DMAs split across the two HWDGE queues
        # (sync + scalar) with the first wq / xq chunk pair shipped first,
        # so the first projection matmul can start as early as possible ---
        wq_r = wq_d[:].rearrange("p (c h) -> p c h", c=DC)
        xq_t = xpool.tile([P, DC, SB], qkdt, name="xq0", tag="xs")
        xk_t = xpool.tile([P, DC, SB], qkdt, name="xk0", tag="xs")
        xv_t = xpool.tile([P, DC, SB], mmdt, name="xv0", tag="xs")
        nc.sync.dma_start(wq_sb[:, 0:2], wq_r[:, 0:2])
        nc.scalar.dma_start(xq_t[:, 0:2], xq_r[:, 0, 0:2])
        nc.sync.dma_start(wq_sb[:, 2:DC], wq_r[:, 2:DC])
        nc.scalar.dma_start(xq_t[:, 2:DC], xq_r[:, 0, 2:DC])
        nc.sync.dma_start(wk_sb[:], wk_d[:].rearrange("p (c h) -> p c h", c=DC))
        nc.scalar.dma_start(xk_t[:], xk_r[:, 0])
        nc.sync.dma_start(xv_t[:], xv_r[:, 0])
        nc.scalar.dma_start(bq_sb[:], bq_d[:])
        nc.scalar.dma_start(bk_sb[:], bk_d[:])
        nc.sync.dma_start(wv_sb[:], wv_d[:].rearrange("p (c h) -> p c h",
                                                      c=DC))
        nc.scalar.dma_start(bv_row[:], bv_d[:])
        nc.sync.dma_start(wo_sb[:], wo_d[:].rearrange("p (c o) -> p c o",
                                                      c=HC))
        xt = {"q": xq_t, "k": xk_t, "v": xv_t}

        nc.gpsimd.partition_broadcast(bv_bc[:], bv_row[:])
        ones_f = consts.tile([P, SCK * NH], f32)
        nc.gpsimd.memset(ones_f[:], 1.0)
        nc.vector.tensor_copy(
            v_aug[:, :, :, DK],
            ones_f[:].rearrange("p (a b) -> p a b", a=SCK))
        for m in range(2):
            nc.gpsimd.memset(masks[m][:], 1.0)
            if causal:
                nc.gpsimd.affine_select(
                    masks[m][:], masks[m][:], pattern=[[-P, G], [1, SB]],
                    compare_op=is_ge, fill=0.0, base=-P * (G * m),
                    channel_multiplier=-1)

        bv_bc_r = bv_bc[:].rearrange("p (h e) -> p h e", h=NH)

        def proj_qk(sbi, xt):
            ss = slice(sbi * SB, (sbi + 1) * SB)
            # hc-major: head 0's q and k biases complete first, so its
            # score matmuls can start earliest
            for hc in range(HC):
                for name, w_sb, b_sb, t_sb in (
                        ("q", wq_sb, bq_sb, q_sb), ("k", wk_sb, bk_sb, k_sb)):
                    x = xt[name]
                    ps = proj_ps.tile([P, SB], f32, name="proj", tag="proj")
                    if qk_fp8:
                        # fp8 DoubleRow: two 128-deep k-subtiles per matmul
                        for j in range(DC // 2):
                            nc.tensor.matmul(
                                ps[:],
                                w_sb[:, 2 * j:2 * j + 2, hc * P:(hc + 1) * P],
                                x[:, 2 * j:2 * j + 2, :], perf_mode=DR,
                                start=(j == 0), stop=(j == DC // 2 - 1))
                    else:
                        for dc in range(DC):
                            nc.tensor.matmul(
                                ps[:], w_sb[:, dc, hc * P:(hc + 1) * P],
                                x[:, dc, :], start=(dc == 0),
                                stop=(dc == DC - 1))
                    nc.vector.tensor_scalar_add(
                        t_sb[:, hc, ss], ps[:], b_sb[:, hc:hc + 1])

        def vproj_fillers(sbi, xt):
            """One filler per 128-seq chunk of the v projection."""
            x = xt["v"]

            def mk(scl):
                def run():
                    sc = sbi * (SB // P) + scl
                    ps = proj_ps.tile([P, DHL], f32, name="proj", tag="proj")
                    for dc in range(DC):
                        nc.tensor.matmul(
                            ps[:], x[:, dc, scl * P:(scl + 1) * P],
                            wv_sb[:, dc, :], start=(dc == 0),
                            stop=(dc == DC - 1))
                    nc.vector.tensor_add(
                        v_aug[:, sc, :, 0:DK],
                        ps[:].rearrange("p (h e) -> p h e", h=NH), bv_bc_r)
                return run
            return [mk(scl) for scl in range(SB // P)]

        def outproj_fillers(sbi, last=False):
            """One filler per output-dim chunk of the out projection."""
            ss = slice(sbi * SB, (sbi + 1) * SB)
            o_sb = opool.tile([P, DC, SB], mmdt, name=f"o{sbi}", tag="ot")

            def mk(oc):
                def run():
                    ps = proj_ps.tile([P, SB], f32, name="proj", tag="proj")
                    for hc in range(HC):
                        nc.tensor.matmul(
                            ps[:], wo_sb[:, hc, oc * P:(oc + 1) * P],
                            attn_sb[:, hc, ss], start=(hc == 0),
                            stop=(hc == HC - 1))
                    # psum -> sbuf bf16 staging; vector during attention
                    # (scalar is saturated by exp), alternated on the tail
                    if last and oc % 2 == 0:
                        nc.scalar.copy(o_sb[:, oc, :], ps[:])
                    else:
                        nc.vector.tensor_copy(o_sb[:, oc, :], ps[:])
                    # ship each half as soon as its copies land; on the
                    # final block, quarter granularity shortens the tail
                    if last:
                        if oc % 2 == 1:
                            nc.sync.dma_start(out_r[:, sbi, oc - 1:oc + 1],
                                              o_sb[:, oc - 1:oc + 1])
                    elif oc == DC // 2 - 1:
                        nc.sync.dma_start(out_r[:, sbi, 0:DC // 2],
                                          o_sb[:, 0:DC // 2])
                    elif oc == DC - 1:
                        nc.sync.dma_start(out_r[:, sbi, DC // 2:DC],
                                          o_sb[:, DC // 2:DC])
                return run
            return [mk(oc) for oc in range(DC)]

        def attn(qb, v_fill, fillers):
            """Attention for query block qb.  v_fill are the v-projection
            fillers for THIS block (issued inside head 0, just before its
            diagonal PV, so the scalar engine gets head 0's scores to exp
            during the q/k projections).  fillers (previous block's out
            projection) are issued as two coarse lumps inside heads 1 and
            2 at the same point, keeping the scalar engine fed across the
            whole block while preserving long same-kind matmul runs."""
            n_chunks = (qb + 1) * (SB // P) if causal else SCK
            n_groups = n_chunks // G
            lumps = {0: list(v_fill),
                     1: list(fillers[:len(fillers) // 2]),
                     2: list(fillers[len(fillers) // 2:])}

            for hl in range(NH):
                hc = hl // HPC
                po = (hl % HPC) * DK
                q_slice = q_sb[po:po + DK, hc, qb * SB:(qb + 1) * SB]
                pv = pv_ps_pool.tile([DK + 1, SB], f32, name="pv", tag="pv")

                def issue_sc(g):
                    sc_t = sc_ps_pool.tile([P, G, SB], f32, name="sc",
                                           tag="sc")
                    for j2 in range(G):
                        tj = g * G + j2
                        nc.tensor.matmul(
                            sc_t[:, j2, :],
                            k_sb[po:po + DK, hc, tj * P:(tj + 1) * P],
                            q_slice, start=True, stop=True)
                    return sc_t

                sc_cur = issue_sc(0)
                for g in range(n_groups):
                    diag = causal and g >= n_groups - 2
                    # software pipeline: issue next group's score matmuls
                    # before this group's PV so the PE never waits on exp
                    sc_next = issue_sc(g + 1) if g + 1 < n_groups else None
                    ex = exp_pool.tile([P, G, SB], mmdt, name="ex", tag="ex")
                    nc.scalar.activation(ex[:], sc_cur[:], Exp, bias=0.0,
                                         scale=inv_sqrt_dk)
                    if diag:
                        nc.vector.tensor_mul(
                            ex[:], ex[:], masks[g - (n_groups - 2)][:])
                    if g == n_groups - 2 and hl in lumps:
                        for f in lumps.pop(hl):
                            f()
                    for j2 in range(G):
                        tj = g * G + j2
                        nc.tensor.matmul(
                            pv[:], v_aug[:, tj, hl, :], ex[:, j2, :],
                            start=(g == 0 and j2 == 0),
                            stop=(g == n_groups - 1 and j2 == G - 1))
                    sc_cur = sc_next
                den = small.tile([1, SB], f32, name="den", tag="den")
                nc.vector.tensor_copy(den[:], pv[DK:DK + 1, :])
                recip = small.tile([1, SB], f32, name="recip", tag="recip")
                nc.vector.reciprocal_approx_fast(recip[:], den[:])
                recip_bc = small.tile([DK, SB], f32, name="recip_bc",
                                      tag="rbc")
                nc.gpsimd.partition_broadcast(recip_bc[:], recip[:])
                nc.vector.tensor_mul(
                    attn_sb[po:po + DK, hc, qb * SB:(qb + 1) * SB],
                    pv[0:DK, :], recip_bc[:])
            for lump in lumps.values():
                for f in lump:
                    f()

        if causal:
            # --- interleaved schedule: per query block, q/k projections are
            # issued first, then attention with the v projection of this
            # block and the out projection of the previous block woven in
            # as fillers
            proj_qk(0, xt)
            vf = vproj_fillers(0, xt)
            xt = load_x(1)
            attn(0, vf, [])
            for sbi in range(1, NSB):
                proj_qk(sbi, xt)
                vf = vproj_fillers(sbi, xt)
                xt = load_x(sbi + 1) if sbi + 1 < NSB else None
                attn(sbi, vf, outproj_fillers(sbi - 1))
            for f in outproj_fillers(NSB - 1, last=True):
                f()
        else:
            # dense attention reads every key block, so all projections
            # must be issued before any attention
            for sbi in range(NSB):
                proj_qk(sbi, xt)
                for f in vproj_fillers(sbi, xt):
                    f()
                xt = load_x(sbi + 1) if sbi + 1 < NSB else None
            for qb in range(NSB):
                attn(qb, [], outproj_fillers(qb - 1) if qb else [])
            for f in outproj_fillers(NSB - 1, last=True):
                f()

    nc.compile()
    return nc


def _get_nc(causal):
    key = ("causal" if causal else "dense", QK_FP8)
    if key not in _CACHE:
        _CACHE[key] = _build_nc(causal, QK_FP8)
    return _CACHE[key]


def _arrange_x(x, dt_):
    """[S, D] f32 -> [P, NSB*DC*SB] with layout [p, sbi, dc, s']."""
    a = x.T.reshape(DC, P, NSB, SB).transpose(1, 2, 0, 3)
    return np.ascontiguousarray(a.reshape(P, NSB * DC * SB)).astype(dt_)


def _prep_core_inputs(Q, K, V, Wq, bq, Wk, bk, Wv, bv, Wo):
    """Build the 8 per-core input maps (all arrays C-contiguous)."""
    import ml_dtypes
    bf16 = ml_dtypes.bfloat16
    qk_dt = ml_dtypes.float8_e4m3 if QK_FP8 else bf16
    cc = np.ascontiguousarray
    x_arr = {}
    for b in range(B):
        x_arr[("q", b)] = _arrange_x(Q[b], qk_dt)
        x_arr[("k", b)] = _arrange_x(K[b], qk_dt)
        x_arr[("v", b)] = _arrange_x(V[b], bf16)
    in_maps = []
    for c in range(NCORES):
        b = c // NGROUPS
        g = c % NGROUPS
        hs, he = g * DHL, (g + 1) * DHL
        # weights pre-arranged to SBUF layout [128, DC, DHL] with d = dc*128+p
        wq_a = cc(Wq[hs:he, :].T.reshape(DC, P, DHL).transpose(1, 0, 2)
                  .reshape(P, DC * DHL))
        wk_a = cc(Wk[hs:he, :].T.reshape(DC, P, DHL).transpose(1, 0, 2)
                  .reshape(P, DC * DHL))
        wv_a = cc(Wv[hs:he, :].T.reshape(DC, P, DHL).transpose(1, 0, 2)
                  .reshape(P, DC * DHL))
        # Wo shard: lhsT layout [hd, dout] split to [128, HC, D], hd = hc*128+p
        wo_a = cc(Wo[:, hs:he].T.reshape(HC, P, D).transpose(1, 0, 2)
                  .reshape(P, HC * D))
        in_maps.append({
            "xq_a": x_arr[("q", b)], "xk_a": x_arr[("k", b)],
            "xv_a": x_arr[("v", b)],
            "wq_a": wq_a.astype(qk_dt), "wk_a": wk_a.astype(qk_dt),
            "wv_a": wv_a.astype(bf16), "wo_a": wo_a.astype(bf16),
            "bq_a": cc(bq[hs:he].reshape(HC, P).T),
            "bk_a": cc(bk[hs:he].reshape(HC, P).T),
            "bv_a": cc(bv[hs:he].reshape(1, DHL)),
        })
    return in_maps


def _classify_mask(mask):
    m = np.asarray(mask)
    if m.dtype != np.bool_:
        m = m.astype(bool)
    causal = np.tril(np.ones((S, S), dtype=bool))
    if all(np.array_equal(m[b, 0], causal) for b in range(m.shape[0])):
        return "causal"
    if m.all():
        return "dense"
    return "generic"


def _numpy_reference(Q, K, V, mask, Wq, bq, Wk, bk, Wv, bv, Wo, bo):
    """Float64-free plain numpy fallback for arbitrary masks."""
    out = np.empty((B, S, D), dtype=np.float32)
    for b in range(B):
        q = (Q[b] @ Wq.T + bq).reshape(S, H, DK).transpose(1, 0, 2)
        k = (K[b] @ Wk.T + bk).reshape(S, H, DK).transpose(1, 0, 2)
        v = (V[b] @ Wv.T + bv).reshape(S, H, DK).transpose(1, 0, 2)
        m = np.asarray(mask[b, 0], dtype=bool)
        acc = np.empty((H, S, DK), dtype=np.float32)
        for h in range(H):
            s = (q[h] @ k[h].T) / np.float32(np.sqrt(DK))
            s = np.where(m, s, np.float32(-1e9))
            s = s - s.max(axis=-1, keepdims=True)
            e = np.exp(s)
            p = e / e.sum(axis=-1, keepdims=True)
            acc[h] = p @ v[h]
        out[b] = acc.transpose(1, 0, 2).reshape(S, D) @ Wo.T + bo
    return out


def _unarrange_out(a):
    """[P, NSB*DC*SB] -> [S, D] f32."""
    t = a.reshape(P, NSB, DC, SB).astype(np.float32)
    return t.transpose(1, 3, 2, 0).reshape(S, D)


def kernel(Q, K, V, mask, Wq, bq, Wk, bk, Wv, bv, Wo, bo,
           _profile=False, _trace_dir=None):
    from concourse.bass_utils import run_bass_kernel_spmd

    flavor = _classify_mask(mask)
    if flavor == "generic":
        return _numpy_reference(Q, K, V, mask, Wq, bq, Wk, bk, Wv, bv, Wo, bo)

    nc = _get_nc(flavor == "causal")
    in_maps = _prep_core_inputs(
        np.asarray(Q, np.float32), np.asarray(K, np.float32),
        np.asarray(V, np.float32), np.asarray(Wq, np.float32),
        np.asarray(bq, np.float32), np.asarray(Wk, np.float32),
        np.asarray(bk, np.float32), np.asarray(Wv, np.float32),
        np.asarray(bv, np.float32), np.asarray(Wo, np.float32))

    kwargs = {}
    if _profile:
        import types
        if "antenv.axon_hooks" not in sys.modules:
            _mod = types.ModuleType("antenv.axon_hooks")
            _mod._hook = None
            _mod.set_axon_ntff_profile_hook = (
                lambda h, _m=_mod: setattr(_m, "_hook", h))
            _mod.get_axon_ntff_profile_hook = lambda _m=_mod: _m._hook
            sys.modules["antenv.axon_hooks"] = _mod
            try:
                import antenv
                antenv.axon_hooks = _mod
            except ImportError:
                pass
        _mod = sys.modules["antenv.axon_hooks"]
        if _mod.get_axon_ntff_profile_hook() is None:
            from trn_agent_boot.trn_boot import _ntff_profile_via_ctypes
            _mod.set_axon_ntff_profile_hook(
                _ntff_profile_via_ctypes("/opt/axon/libaxon_pjrt.so"))
        import concourse.bass_utils as _bu
        _bu.upload_artifacts = lambda d: d  # no cloud copy in this container
        kwargs = dict(trace=True, trace_cores=[0])
        if _trace_dir is not None:
            kwargs["tmpdir"] = _trace_dir
    res = run_bass_kernel_spmd(nc, in_maps, core_ids=list(range(NCORES)),
                               **kwargs)

    out = np.empty((B, S, D), dtype=np.float32)
    bo32 = np.asarray(bo, np.float32)
    for b in range(B):
        acc = _unarrange_out(np.asarray(res.results[b * NGROUPS]["out_a"]))
        for g in range(1, NGROUPS):
            acc = acc + _unarrange_out(
                np.asarray(res.results[b * NGROUPS + g]["out_a"]))
        out[b] = acc + bo32
    if _profile:
        kernel._last_exec_time_ns = res.exec_time_ns
        kernel._last_results = res
    return out


# revision 28
# speedup vs baseline: 1.0119x; 1.0119x over previous
"""Multi-head causal attention (B=2, S=2048, D=1024, H=16) on 8 Trainium2
NeuronCores.

Sharding: data-parallel over the 2 batches x tensor-parallel over 4 head
groups (4 heads each).  Core c handles batch c//4, heads [4*(c%4), 4*(c%4)+4).
Each core computes its Q/K/V projections from column shards of Wq/Wk/Wv,
runs causal attention for its heads, and applies its row shard of Wo,
producing a partial (D, S) output in bf16.  The host sums the 4 partials
per batch in f32 and adds the output bias.

On-core layout: activations are kept transposed (feature dim on SBUF
partitions, sequence on the free axis) so every matmul's operands are
already in the (K x M)/(K x N) form the PE array wants, and the softmax
denominator comes free from an extra ones-row appended to V.

Schedule: phases are interleaved per 512-query block
(proj(0), attn(0), proj(1), outproj(0), attn(1), ...) and the attention
inner loop issues score matmuls one group ahead of the PV matmuls, so the
tensor engine never waits on the scalar-engine exp and stays continuously
busy (keeps the PE at its top p-state clock).  The causal mask is applied
by multiplying the exp'd scores with precomputed bf16 0/1 tiles on the
vector engine; softmax division uses the fast approximate reciprocal.
"""

import sys

sys.path.insert(0, "/opt/trn_rl_repo")

import numpy as np

B, S, D, H = 2, 2048, 1024, 16
DK = D // H            # 64 head dim
NCORES = 8
NGROUPS = 4            # head groups (tensor parallel)
NH = H // NGROUPS      # 4 heads per core
DHL = NH * DK          # 256 local head dims per core
P = 128
DC = D // P            # 8 contraction chunks over D
HC = DHL // P          # 2 local head-dim chunks
SB = 512               # query block (matmul moving free size)
NSB = S // SB          # 4
SCK = S // P           # 16 key chunks
HPC = P // DK          # 2 heads per head-dim chunk
G = 2                  # score chunks per exp group (2 PSUM banks)

# fp8(e4m3) DoubleRow matmuls for the Q/K projections (V and the output
# projection stay bf16 — fp8 there fails the 2e-2 accuracy gate).
QK_FP8 = True

_CACHE = {}


def _build_nc(causal, qk_fp8):
    import concourse.bass as bass
    import concourse.bacc as bacc
    import concourse.mybir as mybir
    import concourse.tile as tile
    from contextlib import ExitStack

    f32 = mybir.dt.float32
    mmdt = mybir.dt.bfloat16
    qkdt = mybir.dt.float8e4 if qk_fp8 else mmdt
    DR = mybir.MatmulPerfMode.DoubleRow if qk_fp8 else None
    Exp = mybir.ActivationFunctionType.Exp
    Identity = mybir.ActivationFunctionType.Identity
    is_ge = mybir.AluOpType.is_ge

    nc = bacc.Bacc(None, target_bir_lowering=False, debug=False)

    # x pre-arranged on host to [P, NSB, DC, SB]: per-partition contiguous
    # 8KB blocks per sequence block -> clean 2D DMA.
    xq_d = nc.dram_tensor("xq_a", [P, NSB * DC * SB], qkdt, kind="ExternalInput")
    xk_d = nc.dram_tensor("xk_a", [P, NSB * DC * SB], qkdt, kind="ExternalInput")
    xv_d = nc.dram_tensor("xv_a", [P, NSB * DC * SB], mmdt, kind="ExternalInput")
    # weights pre-arranged on host to the exact SBUF layouts
    wq_d = nc.dram_tensor("wq_a", [P, DC * DHL], qkdt, kind="ExternalInput")
    wk_d = nc.dram_tensor("wk_a", [P, DC * DHL], qkdt, kind="ExternalInput")
    wv_d = nc.dram_tensor("wv_a", [P, DC * DHL], mmdt, kind="ExternalInput")
    wo_d = nc.dram_tensor("wo_a", [P, HC * D], mmdt, kind="ExternalInput")
    bq_d = nc.dram_tensor("bq_a", [P, HC], f32, kind="ExternalInput")
    bk_d = nc.dram_tensor("bk_a", [P, HC], f32, kind="ExternalInput")
    bv_d = nc.dram_tensor("bv_a", [1, DHL], f32, kind="ExternalInput")
    # bf16 partial output, same [P, NSB, DC, SB] arrangement
    out_d = nc.dram_tensor("out_a", [P, NSB * DC * SB], mmdt,
                           kind="ExternalOutput")

    xq_r = xq_d[:].rearrange("p (b c s) -> p b c s", b=NSB, c=DC)
    xk_r = xk_d[:].rearrange("p (b c s) -> p b c s", b=NSB, c=DC)
    xv_r = xv_d[:].rearrange("p (b c s) -> p b c s", b=NSB, c=DC)
    out_r = out_d[:].rearrange("p (b c s) -> p b c s", b=NSB, c=DC)

    inv_sqrt_dk = 1.0 / float(np.sqrt(DK))

    with tile.TileContext(nc) as tc, ExitStack() as ctx:
        consts = ctx.enter_context(tc.tile_pool(name="consts", bufs=1))
        xpool = ctx.enter_context(tc.tile_pool(name="xpool", bufs=6))
        exp_pool = ctx.enter_context(tc.tile_pool(name="exp_pool", bufs=3))
        small = ctx.enter_context(tc.tile_pool(name="small", bufs=6))
        opool = ctx.enter_context(tc.tile_pool(name="opool", bufs=2))
        proj_ps = ctx.enter_context(
            tc.tile_pool(name="proj_ps", bufs=2, space="PSUM"))
        sc_ps_pool = ctx.enter_context(
            tc.tile_pool(name="sc_ps", bufs=2, space="PSUM"))
        pv_ps_pool = ctx.enter_context(
            tc.tile_pool(name="pv_ps", bufs=2, space="PSUM"))

        # --- resident tensors ---
        wq_sb = consts.tile([P, DC, DHL], qkdt)
        wk_sb = consts.tile([P, DC, DHL], qkdt)
        wv_sb = consts.tile([P, DC, DHL], mmdt)
        wo_sb = consts.tile([P, HC, D], mmdt)
        bq_sb = consts.tile([P, HC], f32)
        bk_sb = consts.tile([P, HC], f32)
        bv_row = consts.tile([1, DHL], f32)
        bv_bc = consts.tile([P, DHL], f32)
        q_sb = consts.tile([P, HC, S], mmdt)
        k_sb = consts.tile([P, HC, S], mmdt)
        v_aug = consts.tile([P, SCK, NH, DK + 1], mmdt)
        attn_sb = consts.tile([P, HC, S], mmdt)
        # causal 0/1 masks for the two diagonal groups: plane j2 of mask
        # tile m keeps (p, i) when i - p - 128*(2*m + j2) >= 0
        masks = [consts.tile([P, G, SB], mmdt, name=f"mask{m}")
                 for m in range(2)]

        def load_x(sbi, split=False):
            tiles = {}
            for name, src, dt_ in (("q", xq_r, qkdt), ("k", xk_r, qkdt),
                                   ("v", xv_r, mmdt)):
                t = xpool.tile([P, DC, SB], dt_, name=f"x{name}{sbi}",
                               tag="xs")
                # at startup, route xv over the scalar HWDGE queue so it
                # overlaps the q/k loads on the sync queue
                eng = nc.scalar if (split and name == "v") else nc.sync
                eng.dma_start(t[:], src[:, sbi])
                tiles[name] = t
            return tiles

        # --- startup: weight DMAs split across the two HWDGE queues
        # (sync + scalar) with the first wq / xq chunk pair shipped first,
        # so the first projection matmul can start as early as possible ---
        wq_r = wq_d[:].rearrange("p (c h) -> p c h", c=DC)
        xq_t = xpool.tile([P, DC, SB], qkdt, name="xq0", tag="xs")
        xk_t = xpool.tile([P, DC, SB], qkdt, name="xk0", tag="xs")
        xv_t = xpool.tile([P, DC, SB], mmdt, name="xv0", tag="xs")
        nc.sync.dma_start(wq_sb[:, 0:2], wq_r[:, 0:2])
        nc.scalar.dma_start(xq_t[:, 0:2], xq_r[:, 0, 0:2])
        nc.sync.dma_start(wq_sb[:, 2:DC], wq_r[:, 2:DC])
        nc.scalar.dma_start(xq_t[:, 2:DC], xq_r[:, 0, 2:DC])
        nc.sync.dma_start(wk_sb[:], wk_d[:].rearrange("p (c h) -> p c h", c=DC))
        nc.scalar.dma_start(xk_t[:], xk_r[:, 0])
        nc.sync.dma_start(xv_t[:], xv_r[:, 0])
        nc.scalar.dma_start(bq_sb[:], bq_d[:])
        nc.scalar.dma_start(bk_sb[:], bk_d[:])
        nc.sync.dma_start(wv_sb[:], wv_d[:].rearrange("p (c h) -> p c h",
                                                      c=DC))
        nc.scalar.dma_start(bv_row[:], bv_d[:])
        nc.scalar.dma_start(wo_sb[:], wo_d[:].rearrange("p (c o) -> p c o",
                                                        c=HC))
        xt = {"q": xq_t, "k": xk_t, "v": xv_t}

        nc.gpsimd.partition_broadcast(bv_bc[:], bv_row[:])
        ones_f = consts.tile([P, SCK * NH], f32)
        nc.gpsimd.memset(ones_f[:], 1.0)
        nc.vector.tensor_copy(
            v_aug[:, :, :, DK],
            ones_f[:].rearrange("p (a b) -> p a b", a=SCK))
        for m in range(2):
            nc.gpsimd.memset(masks[m][:], 1.0)
            if causal:
                nc.gpsimd.affine_select(
                    masks[m][:], masks[m][:], pattern=[[-P, G], [1, SB]],
                    compare_op=is_ge, fill=0.0, base=-P * (G * m),
                    channel_multiplier=-1)

        bv_bc_r = bv_bc[:].rearrange("p (h e) -> p h e", h=NH)

        def proj_qk(sbi, xt):
            ss = slice(sbi * SB, (sbi + 1) * SB)
            # hc-major: head 0's q and k biases complete first, so its
            # score matmuls can start earliest
            for hc in range(HC):
                for name, w_sb, b_sb, t_sb in (
                        ("q", wq_sb, bq_sb, q_sb), ("k", wk_sb, bk_sb, k_sb)):
                    x = xt[name]
                    ps = proj_ps.tile([P, SB], f32, name="proj", tag="proj")
                    if qk_fp8:
                        # fp8 DoubleRow: two 128-deep k-subtiles per matmul
                        for j in range(DC // 2):
                            nc.tensor.matmul(
                                ps[:],
                                w_sb[:, 2 * j:2 * j + 2, hc * P:(hc + 1) * P],
                                x[:, 2 * j:2 * j + 2, :], perf_mode=DR,
                                start=(j == 0), stop=(j == DC // 2 - 1))
                    else:
                        for dc in range(DC):
                            nc.tensor.matmul(
                                ps[:], w_sb[:, dc, hc * P:(hc + 1) * P],
                                x[:, dc, :], start=(dc == 0),
                                stop=(dc == DC - 1))
                    if hc == 0 and sbi > 0:
                        # the first head-chunk's biases gate the next
                        # attention block's score matmuls; the scalar
                        # engine is idle at block boundaries while the
                        # vector engine still drains the previous tail
                        nc.scalar.activation(
                            t_sb[:, hc, ss], ps[:], Identity,
                            bias=b_sb[:, hc:hc + 1], scale=1.0)
                    else:
                        nc.vector.tensor_scalar_add(
                            t_sb[:, hc, ss], ps[:], b_sb[:, hc:hc + 1])

        def vproj_fillers(sbi, xt):
            """One filler per 128-seq chunk of the v projection."""
            x = xt["v"]

            def mk(scl):
                def run():
                    sc = sbi * (SB // P) + scl
                    ps = proj_ps.tile([P, DHL], f32, name="proj", tag="proj")
                    for dc in range(DC):
                        nc.tensor.matmul(
                            ps[:], x[:, dc, scl * P:(scl + 1) * P],
                            wv_sb[:, dc, :], start=(dc == 0),
                            stop=(dc == DC - 1))
                    nc.vector.tensor_add(
                        v_aug[:, sc, :, 0:DK],
                        ps[:].rearrange("p (h e) -> p h e", h=NH), bv_bc_r)
                return run
            return [mk(scl) for scl in range(SB // P)]

        def outproj_fillers(sbi, last=False):
            """One filler per output-dim chunk of the out projection."""
            ss = slice(sbi * SB, (sbi + 1) * SB)
            o_sb = opool.tile([P, DC, SB], mmdt, name=f"o{sbi}", tag="ot")

            def mk(oc):
                def run():
                    ps = proj_ps.tile([P, SB], f32, name="proj", tag="proj")
                    for hc in range(HC):
                        nc.tensor.matmul(
                            ps[:], wo_sb[:, hc, oc * P:(oc + 1) * P],
                            attn_sb[:, hc, ss], start=(hc == 0),
                            stop=(hc == HC - 1))
                    # psum -> sbuf bf16 staging; vector during attention
                    # (scalar is saturated by exp), alternated on the tail
                    if last and oc % 2 == 0:
                        nc.scalar.copy(o_sb[:, oc, :], ps[:])
                    else:
                        nc.vector.tensor_copy(o_sb[:, oc, :], ps[:])
                    # ship each half as soon as its copies land; on the
                    # final block, quarter granularity shortens the tail
                    if last:
                        if oc % 2 == 1:
                            nc.sync.dma_start(out_r[:, sbi, oc - 1:oc + 1],
                                              o_sb[:, oc - 1:oc + 1])
                    elif oc == DC // 2 - 1:
                        nc.sync.dma_start(out_r[:, sbi, 0:DC // 2],
                                          o_sb[:, 0:DC // 2])
                    elif oc == DC - 1:
                        nc.sync.dma_start(out_r[:, sbi, DC // 2:DC],
                                          o_sb[:, DC // 2:DC])
                return run
            return [mk(oc) for oc in range(DC)]

        def attn(qb, v_fill, fillers):
            """Attention for query block qb.  v_fill are the v-projection
            fillers for THIS block (issued inside head 0, just before its
            diagonal PV, so the scalar engine gets head 0's scores to exp
            during the q/k projections).  fillers (previous block's out
            projection) are issued as two coarse lumps inside heads 1 and
            2 at the same point, keeping the scalar engine fed across the
            whole block while preserving long same-kind matmul runs."""
            n_chunks = (qb + 1) * (SB // P) if causal else SCK
            n_groups = n_chunks // G
            lumps = {0: list(v_fill),
                     1: list(fillers[:len(fillers) // 2]),
                     2: list(fillers[len(fillers) // 2:])}

            for hl in range(NH):
                hc = hl // HPC
                po = (hl % HPC) * DK
                q_slice = q_sb[po:po + DK, hc, qb * SB:(qb + 1) * SB]
                pv = pv_ps_pool.tile([DK + 1, SB], f32, name="pv", tag="pv")

                def issue_sc(g):
                    sc_t = sc_ps_pool.tile([P, G, SB], f32, name="sc",
                                           tag="sc")
                    for j2 in range(G):
                        tj = g * G + j2
                        nc.tensor.matmul(
                            sc_t[:, j2, :],
                            k_sb[po:po + DK, hc, tj * P:(tj + 1) * P],
                            q_slice, start=True, stop=True)
                    return sc_t

                sc_cur = issue_sc(0)
                for g in range(n_groups):
                    diag = causal and g >= n_groups - 2
                    # software pipeline: issue next group's score matmuls
                    # before this group's PV so the PE never waits on exp
                    sc_next = issue_sc(g + 1) if g + 1 < n_groups else None
                    ex = exp_pool.tile([P, G, SB], mmdt, name="ex", tag="ex")
                    nc.scalar.activation(ex[:], sc_cur[:], Exp, bias=0.0,
                                         scale=inv_sqrt_dk)
                    if diag:
                        nc.vector.tensor_mul(
                            ex[:], ex[:], masks[g - (n_groups - 2)][:])
                    if g == n_groups - 2 and hl in lumps:
                        for f in lumps.pop(hl):
                            f()
                    for j2 in range(G):
                        tj = g * G + j2
                        nc.tensor.matmul(
                            pv[:], v_aug[:, tj, hl, :], ex[:, j2, :],
                            start=(g == 0 and j2 == 0),
                            stop=(g == n_groups - 1 and j2 == G - 1))
                    sc_cur = sc_next
                den = small.tile([1, SB], f32, name="den", tag="den")
                nc.vector.tensor_copy(den[:], pv[DK:DK + 1, :])
                recip = small.tile([1, SB], f32, name="recip", tag="recip")
                nc.vector.reciprocal_approx_fast(recip[:], den[:])
                recip_bc = small.tile([DK, SB], f32, name="recip_bc",
                                      tag="rbc")
                nc.gpsimd.partition_broadcast(recip_bc[:], recip[:])
                nc.vector.tensor_mul(
                    attn_sb[po:po + DK, hc, qb * SB:(qb + 1) * SB],
                    pv[0:DK, :], recip_bc[:])
            for lump in lumps.values():
                for f in lump:
                    f()

        if causal:
            # --- interleaved schedule: per query block, q/k projections are
            # issued first, then attention with the v projection of this
            # block and the out projection of the previous block woven in
            # as fillers
            proj_qk(0, xt)
            vf = vproj_fillers(0, xt)
            xt = load_x(1)
            attn(0, vf, [])
            for sbi in range(1, NSB):
                proj_qk(sbi, xt)
                vf = vproj_fillers(sbi, xt)
                xt = load_x(sbi + 1) if sbi + 1 < NSB else None
                attn(sbi, vf, outproj_fillers(sbi - 1))
            for f in outproj_fillers(NSB - 1, last=True):
                f()
        else:
            # dense attention reads every key block, so all projections
            # must be issued before any attention
            for sbi in range(NSB):
                proj_qk(sbi, xt)
                for f in vproj_fillers(sbi, xt):
                    f()
                xt = load_x(sbi + 1) if sbi + 1 < NSB else None
            for qb in range(NSB):
                attn(qb, [], outproj_fillers(qb - 1) if qb else [])
            for f in outproj_fillers(NSB - 1, last=True):
                f()

    nc.compile()
    return nc


def _get_nc(causal):
    key = ("causal" if causal else "dense", QK_FP8)
    if key not in _CACHE:
        _CACHE[key] = _build_nc(causal, QK_FP8)
    return _CACHE[key]


def _arrange_x(x, dt_):
    """[S, D] f32 -> [P, NSB*DC*SB] with layout [p, sbi, dc, s']."""
    a = x.T.reshape(DC, P, NSB, SB).transpose(1, 2, 0, 3)
    return np.ascontiguousarray(a.reshape(P, NSB * DC * SB)).astype(dt_)


def _prep_core_inputs(Q, K, V, Wq, bq, Wk, bk, Wv, bv, Wo):
    """Build the 8 per-core input maps (all arrays C-contiguous)."""
    import ml_dtypes
    bf16 = ml_dtypes.bfloat16
    qk_dt = ml_dtypes.float8_e4m3 if QK_FP8 else bf16
    cc = np.ascontiguousarray
    x_arr = {}
    for b in range(B):
        x_arr[("q", b)] = _arrange_x(Q[b], qk_dt)
        x_arr[("k", b)] = _arrange_x(K[b], qk_dt)
        x_arr[("v", b)] = _arrange_x(V[b], bf16)
    in_maps = []
    for c in range(NCORES):
        b = c // NGROUPS
        g = c % NGROUPS
        hs, he = g * DHL, (g + 1) * DHL
        # weights pre-arranged to SBUF layout [128, DC, DHL] with d = dc*128+p
        wq_a = cc(Wq[hs:he, :].T.reshape(DC, P, DHL).transpose(1, 0, 2)
                  .reshape(P, DC * DHL))
        wk_a = cc(Wk[hs:he, :].T.reshape(DC, P, DHL).transpose(1, 0, 2)
                  .reshape(P, DC * DHL))
        wv_a = cc(Wv[hs:he, :].T.reshape(DC, P, DHL).transpose(1, 0, 2)
                  .reshape(P, DC * DHL))
        # Wo shard: lhsT layout [hd, dout] split to [128, HC, D], hd = hc*128+p
        wo_a = cc(Wo[:, hs:he].T.reshape(HC, P, D).transpose(1, 0, 2)
                  .reshape(P, HC * D))
        in_maps.append({
            "xq_a": x_arr[("q", b)], "xk_a": x_arr[("k", b)],
            "xv_a": x_arr[("v", b)],
            "wq_a": wq_a.astype(qk_dt), "wk_a": wk_a.astype(qk_dt),
            "wv_a": wv_a.astype(bf16), "wo_a": wo_a.astype(bf16),
            "bq_a": cc(bq[hs:he].reshape(HC, P).T),
            "bk_a": cc(bk[hs:he].reshape(HC, P).T),
            "bv_a": cc(bv[hs:he].reshape(1, DHL)),
        })
    return in_maps


def _classify_mask(mask):
    m = np.asarray(mask)
    if m.dtype != np.bool_:
        m = m.astype(bool)
    causal = np.tril(np.ones((S, S), dtype=bool))
    if all(np.array_equal(m[b, 0], causal) for b in range(m.shape[0])):
        return "causal"
    if m.all():
        return "dense"
    return "generic"


def _numpy_reference(Q, K, V, mask, Wq, bq, Wk, bk, Wv, bv, Wo, bo):
    """Float64-free plain numpy fallback for arbitrary masks."""
    out = np.empty((B, S, D), dtype=np.float32)
    for b in range(B):
        q = (Q[b] @ Wq.T + bq).reshape(S, H, DK).transpose(1, 0, 2)
        k = (K[b] @ Wk.T + bk).reshape(S, H, DK).transpose(1, 0, 2)
        v = (V[b] @ Wv.T + bv).reshape(S, H, DK).transpose(1, 0, 2)
        m = np.asarray(mask[b, 0], dtype=bool)
        acc = np.empty((H, S, DK), dtype=np.float32)
        for h in range(H):
            s = (q[h] @ k[h].T) / np.float32(np.sqrt(DK))
            s = np.where(m, s, np.float32(-1e9))
            s = s - s.max(axis=-1, keepdims=True)
            e = np.exp(s)
            p = e / e.sum(axis=-1, keepdims=True)
            acc[h] = p @ v[h]
        out[b] = acc.transpose(1, 0, 2).reshape(S, D) @ Wo.T + bo
    return out


def _unarrange_out(a):
    """[P, NSB*DC*SB] -> [S, D] f32."""
    t = a.reshape(P, NSB, DC, SB).astype(np.float32)
    return t.transpose(1, 3, 2, 0).reshape(S, D)


def kernel(Q, K, V, mask, Wq, bq, Wk, bk, Wv, bv, Wo, bo,
           _profile=False, _trace_dir=None):
    from concourse.bass_utils import run_bass_kernel_spmd

    flavor = _classify_mask(mask)
    if flavor == "generic":
        return _numpy_reference(Q, K, V, mask, Wq, bq, Wk, bk, Wv, bv, Wo, bo)

    nc = _get_nc(flavor == "causal")
    in_maps = _prep_core_inputs(
        np.asarray(Q, np.float32), np.asarray(K, np.float32),
        np.asarray(V, np.float32), np.asarray(Wq, np.float32),
        np.asarray(bq, np.float32), np.asarray(Wk, np.float32),
        np.asarray(bk, np.float32), np.asarray(Wv, np.float32),
        np.asarray(bv, np.float32), np.asarray(Wo, np.float32))

    kwargs = {}
    if _profile:
        import types
        if "antenv.axon_hooks" not in sys.modules:
            _mod = types.ModuleType("antenv.axon_hooks")
            _mod._hook = None
            _mod.set_axon_ntff_profile_hook = (
                lambda h, _m=_mod: setattr(_m, "_hook", h))
            _mod.get_axon_ntff_profile_hook = lambda _m=_mod: _m._hook
            sys.modules["antenv.axon_hooks"] = _mod
            try:
                import antenv
                antenv.axon_hooks = _mod
            except ImportError:
                pass
        _mod = sys.modules["antenv.axon_hooks"]
        if _mod.get_axon_ntff_profile_hook() is None:
            from trn_agent_boot.trn_boot import _ntff_profile_via_ctypes
            _mod.set_axon_ntff_profile_hook(
                _ntff_profile_via_ctypes("/opt/axon/libaxon_pjrt.so"))
        import concourse.bass_utils as _bu
        _bu.upload_artifacts = lambda d: d  # no cloud copy in this container
        kwargs = dict(trace=True, trace_cores=[0])
        if _trace_dir is not None:
            kwargs["tmpdir"] = _trace_dir
    res = run_bass_kernel_spmd(nc, in_maps, core_ids=list(range(NCORES)),
                               **kwargs)

    out = np.empty((B, S, D), dtype=np.float32)
    bo32 = np.asarray(bo, np.float32)
    for b in range(B):
        acc = _unarrange_out(np.asarray(res.results[b * NGROUPS]["out_a"]))
        for g in range(1, NGROUPS):
            acc = acc + _unarrange_out(
                np.asarray(res.results[b * NGROUPS + g]["out_a"]))
        out[b] = acc + bo32
    if _profile:
        kernel._last_exec_time_ns = res.exec_time_ns
        kernel._last_results = res
    return out


# revision 29
# speedup vs baseline: 1.0232x; 1.0111x over previous
"""Multi-head causal attention (B=2, S=2048, D=1024, H=16) on 8 Trainium2
NeuronCores.

Sharding: data-parallel over the 2 batches x tensor-parallel over 4 head
groups (4 heads each).  Core c handles batch c//4, heads [4*(c%4), 4*(c%4)+4).
Each core computes its Q/K/V projections from column shards of Wq/Wk/Wv,
runs causal attention for its heads, and applies its row shard of Wo,
producing a partial (D, S) output in bf16.  The host sums the 4 partials
per batch in f32 and adds the output bias.

On-core layout: activations are kept transposed (feature dim on SBUF
partitions, sequence on the free axis) so every matmul's operands are
already in the (K x M)/(K x N) form the PE array wants, and the softmax
denominator comes free from an extra ones-row appended to V.

Schedule: phases are interleaved per 512-query block
(proj(0), attn(0), proj(1), outproj(0), attn(1), ...) and the attention
inner loop issues score matmuls one group ahead of the PV matmuls, so the
tensor engine never waits on the scalar-engine exp and stays continuously
busy (keeps the PE at its top p-state clock).  The causal mask is applied
by multiplying the exp'd scores with precomputed bf16 0/1 tiles on the
vector engine; softmax division uses the fast approximate reciprocal.
"""

import sys

sys.path.insert(0, "/opt/trn_rl_repo")

import numpy as np

B, S, D, H = 2, 2048, 1024, 16
DK = D // H            # 64 head dim
NCORES = 8
NGROUPS = 4            # head groups (tensor parallel)
NH = H // NGROUPS      # 4 heads per core
DHL = NH * DK          # 256 local head dims per core
P = 128
DC = D // P            # 8 contraction chunks over D
HC = DHL // P          # 2 local head-dim chunks
SB = 512               # query block (matmul moving free size)
NSB = S // SB          # 4
SCK = S // P           # 16 key chunks
HPC = P // DK          # 2 heads per head-dim chunk
G = 2                  # score chunks per exp group (2 PSUM banks)

# fp8(e4m3) DoubleRow matmuls for the Q/K projections (V and the output
# projection stay bf16 — fp8 there fails the 2e-2 accuracy gate).
QK_FP8 = True

_CACHE = {}


def _build_nc(causal, qk_fp8):
    import concourse.bass as bass
    import concourse.bacc as bacc
    import concourse.mybir as mybir
    import concourse.tile as tile
    from contextlib import ExitStack

    f32 = mybir.dt.float32
    mmdt = mybir.dt.bfloat16
    qkdt = mybir.dt.float8e4 if qk_fp8 else mmdt
    DR = mybir.MatmulPerfMode.DoubleRow if qk_fp8 else None
    Exp = mybir.ActivationFunctionType.Exp
    Identity = mybir.ActivationFunctionType.Identity
    is_ge = mybir.AluOpType.is_ge

    nc = bacc.Bacc(None, target_bir_lowering=False, debug=False)

    # x pre-arranged on host to [P, NSB, DC, SB]: per-partition contiguous
    # 8KB blocks per sequence block -> clean 2D DMA.
    xq_d = nc.dram_tensor("xq_a", [P, NSB * DC * SB], qkdt, kind="ExternalInput")
    xk_d = nc.dram_tensor("xk_a", [P, NSB * DC * SB], qkdt, kind="ExternalInput")
    xv_d = nc.dram_tensor("xv_a", [P, NSB * DC * SB], mmdt, kind="ExternalInput")
    # weights pre-arranged on host to the exact SBUF layouts
    wq_d = nc.dram_tensor("wq_a", [P, DC * DHL], qkdt, kind="ExternalInput")
    wk_d = nc.dram_tensor("wk_a", [P, DC * DHL], qkdt, kind="ExternalInput")
    wv_d = nc.dram_tensor("wv_a", [P, DC * DHL], mmdt, kind="ExternalInput")
    wo_d = nc.dram_tensor("wo_a", [P, HC * D], mmdt, kind="ExternalInput")
    bq_d = nc.dram_tensor("bq_a", [P, HC], f32, kind="ExternalInput")
    bk_d = nc.dram_tensor("bk_a", [P, HC], f32, kind="ExternalInput")
    bv_d = nc.dram_tensor("bv_a", [1, DHL], f32, kind="ExternalInput")
    # bf16 partial output, same [P, NSB, DC, SB] arrangement
    out_d = nc.dram_tensor("out_a", [P, NSB * DC * SB], mmdt,
                           kind="ExternalOutput")

    xq_r = xq_d[:].rearrange("p (b c s) -> p b c s", b=NSB, c=DC)
    xk_r = xk_d[:].rearrange("p (b c s) -> p b c s", b=NSB, c=DC)
    xv_r = xv_d[:].rearrange("p (b c s) -> p b c s", b=NSB, c=DC)
    out_r = out_d[:].rearrange("p (b c s) -> p b c s", b=NSB, c=DC)

    inv_sqrt_dk = 1.0 / float(np.sqrt(DK))

    with tile.TileContext(nc) as tc, ExitStack() as ctx:
        consts = ctx.enter_context(tc.tile_pool(name="consts", bufs=1))
        xpool = ctx.enter_context(tc.tile_pool(name="xpool", bufs=6))
        exp_pool = ctx.enter_context(tc.tile_pool(name="exp_pool", bufs=3))
        small = ctx.enter_context(tc.tile_pool(name="small", bufs=6))
        opool = ctx.enter_context(tc.tile_pool(name="opool", bufs=2))
        proj_ps = ctx.enter_context(
            tc.tile_pool(name="proj_ps", bufs=2, space="PSUM"))
        sc_ps_pool = ctx.enter_context(
            tc.tile_pool(name="sc_ps", bufs=2, space="PSUM"))
        pv_ps_pool = ctx.enter_context(
            tc.tile_pool(name="pv_ps", bufs=2, space="PSUM"))

        # --- resident tensors ---
        wq_sb = consts.tile([P, DC, DHL], qkdt)
        wk_sb = consts.tile([P, DC, DHL], qkdt)
        wv_sb = consts.tile([P, DC, DHL], mmdt)
        wo_sb = consts.tile([P, HC, D], mmdt)
        bq_sb = consts.tile([P, HC], f32)
        bk_sb = consts.tile([P, HC], f32)
        bv_row = consts.tile([1, DHL], f32)
        bv_bc = consts.tile([P, DHL], f32)
        q_sb = consts.tile([P, HC, S], mmdt)
        k_sb = consts.tile([P, HC, S], mmdt)
        v_aug = consts.tile([P, SCK, NH, DK + 1], mmdt)
        attn_sb = consts.tile([P, HC, S], mmdt)
        # causal 0/1 masks for the two diagonal groups: plane j2 of mask
        # tile m keeps (p, i) when i - p - 128*(2*m + j2) >= 0
        masks = [consts.tile([P, G, SB], mmdt, name=f"mask{m}")
                 for m in range(2)]

        def load_x(sbi, split=False):
            tiles = {}
            for name, src, dt_ in (("q", xq_r, qkdt), ("k", xk_r, qkdt),
                                   ("v", xv_r, mmdt)):
                t = xpool.tile([P, DC, SB], dt_, name=f"x{name}{sbi}",
                               tag="xs")
                # at startup, route xv over the scalar HWDGE queue so it
                # overlaps the q/k loads on the sync queue
                eng = nc.scalar if (split and name == "v") else nc.sync
                eng.dma_start(t[:], src[:, sbi])
                tiles[name] = t
            return tiles

        # --- startup: weight DMAs split across the two HWDGE queues
        # (sync + scalar) with the first wq / xq chunk pair shipped first,
        # so the first projection matmul can start as early as possible ---
        wq_r = wq_d[:].rearrange("p (c h) -> p c h", c=DC)
        xq_t = xpool.tile([P, DC, SB], qkdt, name="xq0", tag="xs")
        xk_t = xpool.tile([P, DC, SB], qkdt, name="xk0", tag="xs")
        xv_t = xpool.tile([P, DC, SB], mmdt, name="xv0", tag="xs")
        nc.sync.dma_start(wq_sb[:, 0:2], wq_r[:, 0:2])
        nc.scalar.dma_start(xq_t[:, 0:2], xq_r[:, 0, 0:2])
        nc.sync.dma_start(wq_sb[:, 2:DC], wq_r[:, 2:DC])
        nc.scalar.dma_start(xq_t[:, 2:DC], xq_r[:, 0, 2:DC])
        nc.sync.dma_start(wk_sb[:], wk_d[:].rearrange("p (c h) -> p c h", c=DC))
        nc.gpsimd.dma_start(xk_t[:], xk_r[:, 0])
        nc.sync.dma_start(xv_t[:], xv_r[:, 0])
        nc.scalar.dma_start(bq_sb[:], bq_d[:])
        nc.scalar.dma_start(bk_sb[:], bk_d[:])
        nc.sync.dma_start(wv_sb[:], wv_d[:].rearrange("p (c h) -> p c h",
                                                      c=DC))
        nc.scalar.dma_start(bv_row[:], bv_d[:])
        nc.scalar.dma_start(wo_sb[:], wo_d[:].rearrange("p (c o) -> p c o",
                                                        c=HC))
        xt = {"q": xq_t, "k": xk_t, "v": xv_t}

        nc.gpsimd.partition_broadcast(bv_bc[:], bv_row[:])
        ones_f = consts.tile([P, SCK * NH], f32)
        nc.gpsimd.memset(ones_f[:], 1.0)
        nc.vector.tensor_copy(
            v_aug[:, :, :, DK],
            ones_f[:].rearrange("p (a b) -> p a b", a=SCK))
        for m in range(2):
            nc.gpsimd.memset(masks[m][:], 1.0)
            if causal:
                nc.gpsimd.affine_select(
                    masks[m][:], masks[m][:], pattern=[[-P, G], [1, SB]],
                    compare_op=is_ge, fill=0.0, base=-P * (G * m),
                    channel_multiplier=-1)

        bv_bc_r = bv_bc[:].rearrange("p (h e) -> p h e", h=NH)

        def proj_qk(sbi, xt):
            ss = slice(sbi * SB, (sbi + 1) * SB)
            # hc-major: head 0's q and k biases complete first, so its
            # score matmuls can start earliest
            for hc in range(HC):
                for name, w_sb, b_sb, t_sb in (
                        ("q", wq_sb, bq_sb, q_sb), ("k", wk_sb, bk_sb, k_sb)):
                    x = xt[name]
                    ps = proj_ps.tile([P, SB], f32, name="proj", tag="proj")
                    if qk_fp8:
                        # fp8 DoubleRow: two 128-deep k-subtiles per matmul
                        for j in range(DC // 2):
                            nc.tensor.matmul(
                                ps[:],
                                w_sb[:, 2 * j:2 * j + 2, hc * P:(hc + 1) * P],
                                x[:, 2 * j:2 * j + 2, :], perf_mode=DR,
                                start=(j == 0), stop=(j == DC // 2 - 1))
                    else:
                        for dc in range(DC):
                            nc.tensor.matmul(
                                ps[:], w_sb[:, dc, hc * P:(hc + 1) * P],
                                x[:, dc, :], start=(dc == 0),
                                stop=(dc == DC - 1))
                    if hc == 0 and sbi > 0:
                        # the first head-chunk's biases gate the next
                        # attention block's score matmuls; the scalar
                        # engine is idle at block boundaries while the
                        # vector engine still drains the previous tail
                        nc.scalar.activation(
                            t_sb[:, hc, ss], ps[:], Identity,
                            bias=b_sb[:, hc:hc + 1], scale=1.0)
                    else:
                        nc.vector.tensor_scalar_add(
                            t_sb[:, hc, ss], ps[:], b_sb[:, hc:hc + 1])

        def vproj_fillers(sbi, xt):
            """One filler per 128-seq chunk of the v projection."""
            x = xt["v"]

            def mk(scl):
                def run():
                    sc = sbi * (SB // P) + scl
                    ps = proj_ps.tile([P, DHL], f32, name="proj", tag="proj")
                    for dc in range(DC):
                        nc.tensor.matmul(
                            ps[:], x[:, dc, scl * P:(scl + 1) * P],
                            wv_sb[:, dc, :], start=(dc == 0),
                            stop=(dc == DC - 1))
                    nc.vector.tensor_add(
                        v_aug[:, sc, :, 0:DK],
                        ps[:].rearrange("p (h e) -> p h e", h=NH), bv_bc_r)
                return run
            return [mk(scl) for scl in range(SB // P)]

        def outproj_fillers(sbi, last=False):
            """One filler per output-dim chunk of the out projection."""
            ss = slice(sbi * SB, (sbi + 1) * SB)
            o_sb = opool.tile([P, DC, SB], mmdt, name=f"o{sbi}", tag="ot")

            def mk(oc):
                def run():
                    ps = proj_ps.tile([P, SB], f32, name="proj", tag="proj")
                    for hc in range(HC):
                        nc.tensor.matmul(
                            ps[:], wo_sb[:, hc, oc * P:(oc + 1) * P],
                            attn_sb[:, hc, ss], start=(hc == 0),
                            stop=(hc == HC - 1))
                    # psum -> sbuf bf16 staging; vector during attention
                    # (scalar is saturated by exp), alternated on the tail
                    if last and oc % 2 == 0:
                        nc.scalar.copy(o_sb[:, oc, :], ps[:])
                    else:
                        nc.vector.tensor_copy(o_sb[:, oc, :], ps[:])
                    # ship each half as soon as its copies land; on the
                    # final block, quarter granularity shortens the tail
                    if last:
                        if oc % 2 == 1:
                            nc.sync.dma_start(out_r[:, sbi, oc - 1:oc + 1],
                                              o_sb[:, oc - 1:oc + 1])
                    elif oc == DC // 2 - 1:
                        nc.sync.dma_start(out_r[:, sbi, 0:DC // 2],
                                          o_sb[:, 0:DC // 2])
                    elif oc == DC - 1:
                        nc.sync.dma_start(out_r[:, sbi, DC // 2:DC],
                                          o_sb[:, DC // 2:DC])
                return run
            return [mk(oc) for oc in range(DC)]

        def attn(qb, v_fill, fillers):
            """Attention for query block qb.  v_fill are the v-projection
            fillers for THIS block (issued inside head 0, just before its
            diagonal PV, so the scalar engine gets head 0's scores to exp
            during the q/k projections).  fillers (previous block's out
            projection) are issued as two coarse lumps inside heads 1 and
            2 at the same point, keeping the scalar engine fed across the
            whole block while preserving long same-kind matmul runs."""
            n_chunks = (qb + 1) * (SB // P) if causal else SCK
            n_groups = n_chunks // G
            lumps = {0: list(v_fill),
                     1: list(fillers[:len(fillers) // 2]),
                     2: list(fillers[len(fillers) // 2:])}

            for hl in range(NH):
                hc = hl // HPC
                po = (hl % HPC) * DK
                q_slice = q_sb[po:po + DK, hc, qb * SB:(qb + 1) * SB]
                pv = pv_ps_pool.tile([DK + 1, SB], f32, name="pv", tag="pv")

                def issue_sc(g):
                    sc_t = sc_ps_pool.tile([P, G, SB], f32, name="sc",
                                           tag="sc")
                    for j2 in range(G):
                        tj = g * G + j2
                        nc.tensor.matmul(
                            sc_t[:, j2, :],
                            k_sb[po:po + DK, hc, tj * P:(tj + 1) * P],
                            q_slice, start=True, stop=True)
                    return sc_t

                sc_cur = issue_sc(0)
                for g in range(n_groups):
                    diag = causal and g >= n_groups - 2
                    # software pipeline: issue next group's score matmuls
                    # before this group's PV so the PE never waits on exp
                    sc_next = issue_sc(g + 1) if g + 1 < n_groups else None
                    ex = exp_pool.tile([P, G, SB], mmdt, name="ex", tag="ex")
                    nc.scalar.activation(ex[:], sc_cur[:], Exp, bias=0.0,
                                         scale=inv_sqrt_dk)
                    if diag:
                        nc.vector.tensor_mul(
                            ex[:], ex[:], masks[g - (n_groups - 2)][:])
                    if g == n_groups - 2 and hl in lumps:
                        for f in lumps.pop(hl):
                            f()
                    for j2 in range(G):
                        tj = g * G + j2
                        nc.tensor.matmul(
                            pv[:], v_aug[:, tj, hl, :], ex[:, j2, :],
                            start=(g == 0 and j2 == 0),
                            stop=(g == n_groups - 1 and j2 == G - 1))
                    sc_cur = sc_next
                den = small.tile([1, SB], f32, name="den", tag="den")
                nc.vector.tensor_copy(den[:], pv[DK:DK + 1, :])
                recip = small.tile([1, SB], f32, name="recip", tag="recip")
                nc.vector.reciprocal_approx_fast(recip[:], den[:])
                recip_bc = small.tile([DK, SB], f32, name="recip_bc",
                                      tag="rbc")
                nc.gpsimd.partition_broadcast(recip_bc[:], recip[:])
                nc.vector.tensor_mul(
                    attn_sb[po:po + DK, hc, qb * SB:(qb + 1) * SB],
                    pv[0:DK, :], recip_bc[:])
            for lump in lumps.values():
                for f in lump:
                    f()

        if causal:
            # --- interleaved schedule: per query block, q/k projections are
            # issued first, then attention with the v projection of this
            # block and the out projection of the previous block woven in
            # as fillers
            proj_qk(0, xt)
            vf = vproj_fillers(0, xt)
            xt = load_x(1)
            attn(0, vf, [])
            for sbi in range(1, NSB):
                proj_qk(sbi, xt)
                vf = vproj_fillers(sbi, xt)
                xt = load_x(sbi + 1) if sbi + 1 < NSB else None
                attn(sbi, vf, outproj_fillers(sbi - 1))
            for f in outproj_fillers(NSB - 1, last=True):
                f()
        else:
            # dense attention reads every key block, so all projections
            # must be issued before any attention
            for sbi in range(NSB):
                proj_qk(sbi, xt)
                for f in vproj_fillers(sbi, xt):
                    f()
                xt = load_x(sbi + 1) if sbi + 1 < NSB else None
            for qb in range(NSB):
                attn(qb, [], outproj_fillers(qb - 1) if qb else [])
            for f in outproj_fillers(NSB - 1, last=True):
                f()

    nc.compile()
    return nc


def _get_nc(causal):
    key = ("causal" if causal else "dense", QK_FP8)
    if key not in _CACHE:
        _CACHE[key] = _build_nc(causal, QK_FP8)
    return _CACHE[key]


def _arrange_x(x, dt_):
    """[S, D] f32 -> [P, NSB*DC*SB] with layout [p, sbi, dc, s']."""
    a = x.T.reshape(DC, P, NSB, SB).transpose(1, 2, 0, 3)
    return np.ascontiguousarray(a.reshape(P, NSB * DC * SB)).astype(dt_)


def _prep_core_inputs(Q, K, V, Wq, bq, Wk, bk, Wv, bv, Wo):
    """Build the 8 per-core input maps (all arrays C-contiguous)."""
    import ml_dtypes
    bf16 = ml_dtypes.bfloat16
    qk_dt = ml_dtypes.float8_e4m3 if QK_FP8 else bf16
    cc = np.ascontiguousarray
    x_arr = {}
    for b in range(B):
        x_arr[("q", b)] = _arrange_x(Q[b], qk_dt)
        x_arr[("k", b)] = _arrange_x(K[b], qk_dt)
        x_arr[("v", b)] = _arrange_x(V[b], bf16)
    in_maps = []
    for c in range(NCORES):
        b = c // NGROUPS
        g = c % NGROUPS
        hs, he = g * DHL, (g + 1) * DHL
        # weights pre-arranged to SBUF layout [128, DC, DHL] with d = dc*128+p
        wq_a = cc(Wq[hs:he, :].T.reshape(DC, P, DHL).transpose(1, 0, 2)
                  .reshape(P, DC * DHL))
        wk_a = cc(Wk[hs:he, :].T.reshape(DC, P, DHL).transpose(1, 0, 2)
                  .reshape(P, DC * DHL))
        wv_a = cc(Wv[hs:he, :].T.reshape(DC, P, DHL).transpose(1, 0, 2)
                  .reshape(P, DC * DHL))
        # Wo shard: lhsT layout [hd, dout] split to [128, HC, D], hd = hc*128+p
        wo_a = cc(Wo[:, hs:he].T.reshape(HC, P, D).transpose(1, 0, 2)
                  .reshape(P, HC * D))
        in_maps.append({
            "xq_a": x_arr[("q", b)], "xk_a": x_arr[("k", b)],
            "xv_a": x_arr[("v", b)],
            "wq_a": wq_a.astype(qk_dt), "wk_a": wk_a.astype(qk_dt),
            "wv_a": wv_a.astype(bf16), "wo_a": wo_a.astype(bf16),
            "bq_a": cc(bq[hs:he].reshape(HC, P).T),
            "bk_a": cc(bk[hs:he].reshape(HC, P).T),
            "bv_a": cc(bv[hs:he].reshape(1, DHL)),
        })
    return in_maps


def _classify_mask(mask):
    m = np.asarray(mask)
    if m.dtype != np.bool_:
        m = m.astype(bool)
    causal = np.tril(np.ones((S, S), dtype=bool))
    if all(np.array_equal(m[b, 0], causal) for b in range(m.shape[0])):
        return "causal"
    if m.all():
        return "dense"
    return "generic"


def _numpy_reference(Q, K, V, mask, Wq, bq, Wk, bk, Wv, bv, Wo, bo):
    """Float64-free plain numpy fallback for arbitrary masks."""
    out = np.empty((B, S, D), dtype=np.float32)
    for b in range(B):
        q = (Q[b] @ Wq.T + bq).reshape(S, H, DK).transpose(1, 0, 2)
        k = (K[b] @ Wk.T + bk).reshape(S, H, DK).transpose(1, 0, 2)
        v = (V[b] @ Wv.T + bv).reshape(S, H, DK).transpose(1, 0, 2)
        m = np.asarray(mask[b, 0], dtype=bool)
        acc = np.empty((H, S, DK), dtype=np.float32)
        for h in range(H):
            s = (q[h] @ k[h].T) / np.float32(np.sqrt(DK))
            s = np.where(m, s, np.float32(-1e9))
            s = s - s.max(axis=-1, keepdims=True)
            e = np.exp(s)
            p = e / e.sum(axis=-1, keepdims=True)
            acc[h] = p @ v[h]
        out[b] = acc.transpose(1, 0, 2).reshape(S, D) @ Wo.T + bo
    return out


def _unarrange_out(a):
    """[P, NSB*DC*SB] -> [S, D] f32."""
    t = a.reshape(P, NSB, DC, SB).astype(np.float32)
    return t.transpose(1, 3, 2, 0).reshape(S, D)


def kernel(Q, K, V, mask, Wq, bq, Wk, bk, Wv, bv, Wo, bo,
           _profile=False, _trace_dir=None):
    from concourse.bass_utils import run_bass_kernel_spmd

    flavor = _classify_mask(mask)
    if flavor == "generic":
        return _numpy_reference(Q, K, V, mask, Wq, bq, Wk, bk, Wv, bv, Wo, bo)

    nc = _get_nc(flavor == "causal")
    in_maps = _prep_core_inputs(
        np.asarray(Q, np.float32), np.asarray(K, np.float32),
        np.asarray(V, np.float32), np.asarray(Wq, np.float32),
        np.asarray(bq, np.float32), np.asarray(Wk, np.float32),
        np.asarray(bk, np.float32), np.asarray(Wv, np.float32),
        np.asarray(bv, np.float32), np.asarray(Wo, np.float32))

    kwargs = {}
    if _profile:
        import types
        if "antenv.axon_hooks" not in sys.modules:
            _mod = types.ModuleType("antenv.axon_hooks")
            _mod._hook = None
            _mod.set_axon_ntff_profile_hook = (
                lambda h, _m=_mod: setattr(_m, "_hook", h))
            _mod.get_axon_ntff_profile_hook = lambda _m=_mod: _m._hook
            sys.modules["antenv.axon_hooks"] = _mod
            try:
                import antenv
                antenv.axon_hooks = _mod
            except ImportError:
                pass
        _mod = sys.modules["antenv.axon_hooks"]
        if _mod.get_axon_ntff_profile_hook() is None:
            from trn_agent_boot.trn_boot import _ntff_profile_via_ctypes
            _mod.set_axon_ntff_profile_hook(
                _ntff_profile_via_ctypes("/opt/axon/libaxon_pjrt.so"))
        import concourse.bass_utils as _bu
        _bu.upload_artifacts = lambda d: d  # no cloud copy in this container
        kwargs = dict(trace=True, trace_cores=[0])
        if _trace_dir is not None:
            kwargs["tmpdir"] = _trace_dir
    res = run_bass_kernel_spmd(nc, in_maps, core_ids=list(range(NCORES)),
                               **kwargs)

    out = np.empty((B, S, D), dtype=np.float32)
    bo32 = np.asarray(bo, np.float32)
    for b in range(B):
        acc = _unarrange_out(np.asarray(res.results[b * NGROUPS]["out_a"]))
        for g in range(1, NGROUPS):
            acc = acc + _unarrange_out(
                np.asarray(res.results[b * NGROUPS + g]["out_a"]))
        out[b] = acc + bo32
    if _profile:
        kernel._last_exec_time_ns = res.exec_time_ns
        kernel._last_results = res
    return out


# revision 32
# speedup vs baseline: 1.0486x; 1.0249x over previous
"""Multi-head causal attention (B=2, S=2048, D=1024, H=16) on 8 Trainium2
NeuronCores.

Sharding: data-parallel over the 2 batches x tensor-parallel over 4 head
groups (4 heads each).  Core c handles batch c//4, heads [4*(c%4), 4*(c%4)+4).
Each core computes its Q/K/V projections from column shards of Wq/Wk/Wv,
runs causal attention for its heads, and applies its row shard of Wo,
producing a partial (D, S) output in bf16.  The host sums the 4 partials
per batch in f32 and adds the output bias.

On-core layout: activations are kept transposed (feature dim on SBUF
partitions, sequence on the free axis) so every matmul's operands are
already in the (K x M)/(K x N) form the PE array wants, and the softmax
denominator comes free from an extra ones-row appended to V.

Schedule: phases are interleaved per 512-query block
(proj(0), attn(0), proj(1), outproj(0), attn(1), ...) and the attention
inner loop issues score matmuls one group ahead of the PV matmuls, so the
tensor engine never waits on the scalar-engine exp and stays continuously
busy (keeps the PE at its top p-state clock).  The causal mask is applied
by multiplying the exp'd scores with precomputed bf16 0/1 tiles on the
vector engine; softmax division uses the fast approximate reciprocal.
"""

import sys

sys.path.insert(0, "/opt/trn_rl_repo")

import numpy as np

B, S, D, H = 2, 2048, 1024, 16
DK = D // H            # 64 head dim
NCORES = 8
NGROUPS = 4            # head groups (tensor parallel)
NH = H // NGROUPS      # 4 heads per core
DHL = NH * DK          # 256 local head dims per core
P = 128
DC = D // P            # 8 contraction chunks over D
HC = DHL // P          # 2 local head-dim chunks
SB = 512               # query block (matmul moving free size)
NSB = S // SB          # 4
SCK = S // P           # 16 key chunks
HPC = P // DK          # 2 heads per head-dim chunk
G = 2                  # score chunks per exp group (2 PSUM banks)

# fp8(e4m3) DoubleRow matmuls for the Q/K projections (V and the output
# projection stay bf16 — fp8 there fails the 2e-2 accuracy gate).
QK_FP8 = True

_CACHE = {}


def _build_nc(causal, qk_fp8):
    import concourse.bass as bass
    import concourse.bacc as bacc
    import concourse.mybir as mybir
    import concourse.tile as tile
    from contextlib import ExitStack

    f32 = mybir.dt.float32
    mmdt = mybir.dt.bfloat16
    qkdt = mybir.dt.float8e4 if qk_fp8 else mmdt
    DR = mybir.MatmulPerfMode.DoubleRow if qk_fp8 else None
    Exp = mybir.ActivationFunctionType.Exp
    Identity = mybir.ActivationFunctionType.Identity
    is_ge = mybir.AluOpType.is_ge

    nc = bacc.Bacc(None, target_bir_lowering=False, debug=False)

    # x pre-arranged on host to [P, NSB, DC, SB]: per-partition contiguous
    # 8KB blocks per sequence block -> clean 2D DMA.
    xq_d = nc.dram_tensor("xq_a", [P, NSB * DC * SB], qkdt, kind="ExternalInput")
    xk_d = nc.dram_tensor("xk_a", [P, NSB * DC * SB], qkdt, kind="ExternalInput")
    xv_d = nc.dram_tensor("xv_a", [P, NSB * DC * SB], mmdt, kind="ExternalInput")
    # weights pre-arranged on host to the exact SBUF layouts
    wq_d = nc.dram_tensor("wq_a", [P, DC * DHL], qkdt, kind="ExternalInput")
    wk_d = nc.dram_tensor("wk_a", [P, DC * DHL], qkdt, kind="ExternalInput")
    wv_d = nc.dram_tensor("wv_a", [P, DC * DHL], mmdt, kind="ExternalInput")
    wo_d = nc.dram_tensor("wo_a", [P, HC * D], mmdt, kind="ExternalInput")
    bq_d = nc.dram_tensor("bq_a", [P, HC], f32, kind="ExternalInput")
    bk_d = nc.dram_tensor("bk_a", [P, HC], f32, kind="ExternalInput")
    bv_d = nc.dram_tensor("bv_a", [1, DHL], f32, kind="ExternalInput")
    # bf16 partial output, same [P, NSB, DC, SB] arrangement
    out_d = nc.dram_tensor("out_a", [P, NSB * DC * SB], mmdt,
                           kind="ExternalOutput")

    xq_r = xq_d[:].rearrange("p (b c s) -> p b c s", b=NSB, c=DC)
    xk_r = xk_d[:].rearrange("p (b c s) -> p b c s", b=NSB, c=DC)
    xv_r = xv_d[:].rearrange("p (b c s) -> p b c s", b=NSB, c=DC)
    out_r = out_d[:].rearrange("p (b c s) -> p b c s", b=NSB, c=DC)

    inv_sqrt_dk = 1.0 / float(np.sqrt(DK))

    with tile.TileContext(nc) as tc, ExitStack() as ctx:
        consts = ctx.enter_context(tc.tile_pool(name="consts", bufs=1))
        xpool = ctx.enter_context(tc.tile_pool(name="xpool", bufs=6))
        exp_pool = ctx.enter_context(tc.tile_pool(name="exp_pool", bufs=3))
        small = ctx.enter_context(tc.tile_pool(name="small", bufs=6))
        opool = ctx.enter_context(tc.tile_pool(name="opool", bufs=2))
        proj_ps = ctx.enter_context(
            tc.tile_pool(name="proj_ps", bufs=2, space="PSUM"))
        sc_ps_pool = ctx.enter_context(
            tc.tile_pool(name="sc_ps", bufs=2, space="PSUM"))
        pv_ps_pool = ctx.enter_context(
            tc.tile_pool(name="pv_ps", bufs=2, space="PSUM"))

        # --- resident tensors ---
        wq_sb = consts.tile([P, DC, DHL], qkdt)
        wk_sb = consts.tile([P, DC, DHL], qkdt)
        wv_sb = consts.tile([P, DC, DHL], mmdt)
        wo_sb = consts.tile([P, HC, D], mmdt)
        bq_sb = consts.tile([P, HC], f32)
        bk_sb = consts.tile([P, HC], f32)
        bv_row = consts.tile([1, DHL], f32)
        bv_bc = consts.tile([P, DHL], f32)
        q_sb = consts.tile([P, HC, S], mmdt)
        k_sb = consts.tile([P, HC, S], mmdt)
        v_aug = consts.tile([P, SCK, NH, DK + 1], mmdt)
        attn_sb = consts.tile([P, HC, S], mmdt)
        # causal 0/1 masks for the two diagonal groups: plane j2 of mask
        # tile m keeps (p, i) when i - p - 128*(2*m + j2) >= 0
        masks = [consts.tile([P, G, SB], mmdt, name=f"mask{m}")
                 for m in range(2)]
        # half-block mask for the split diagonal: keeps (p, j2, i') when
        # i' - p - 128*j2 >= 0 over a 256-query half block
        HB = SB // 2
        mask_h = consts.tile([P, G, HB], mmdt, name="mask_h")

        def load_x(sbi, split=False):
            tiles = {}
            for name, src, dt_ in (("q", xq_r, qkdt), ("k", xk_r, qkdt),
                                   ("v", xv_r, mmdt)):
                t = xpool.tile([P, DC, SB], dt_, name=f"x{name}{sbi}",
                               tag="xs")
                # at startup, route xv over the scalar HWDGE queue so it
                # overlaps the q/k loads on the sync queue
                eng = nc.scalar if (split and name == "v") else nc.sync
                eng.dma_start(t[:], src[:, sbi])
                tiles[name] = t
            return tiles

        # --- startup: weight DMAs split across the two HWDGE queues
        # (sync + scalar) with the first wq / xq chunk pair shipped first,
        # so the first projection matmul can start as early as possible ---
        wq_r = wq_d[:].rearrange("p (c h) -> p c h", c=DC)
        xq_t = xpool.tile([P, DC, SB], qkdt, name="xq0", tag="xs")
        xk_t = xpool.tile([P, DC, SB], qkdt, name="xk0", tag="xs")
        xv_t = xpool.tile([P, DC, SB], mmdt, name="xv0", tag="xs")
        nc.sync.dma_start(wq_sb[:, 0:2], wq_r[:, 0:2])
        nc.scalar.dma_start(xq_t[:, 0:2], xq_r[:, 0, 0:2])
        nc.sync.dma_start(wq_sb[:, 2:DC], wq_r[:, 2:DC])
        nc.scalar.dma_start(xq_t[:, 2:DC], xq_r[:, 0, 2:DC])
        nc.sync.dma_start(wk_sb[:], wk_d[:].rearrange("p (c h) -> p c h", c=DC))
        nc.gpsimd.dma_start(xk_t[:], xk_r[:, 0])
        nc.sync.dma_start(xv_t[:], xv_r[:, 0])
        nc.scalar.dma_start(bq_sb[:], bq_d[:])
        nc.scalar.dma_start(bk_sb[:], bk_d[:])
        nc.sync.dma_start(wv_sb[:], wv_d[:].rearrange("p (c h) -> p c h",
                                                      c=DC))
        nc.scalar.dma_start(bv_row[:], bv_d[:])
        nc.scalar.dma_start(wo_sb[:], wo_d[:].rearrange("p (c o) -> p c o",
                                                        c=HC))
        xt = {"q": xq_t, "k": xk_t, "v": xv_t}

        nc.gpsimd.partition_broadcast(bv_bc[:], bv_row[:])
        ones_f = consts.tile([P, SCK * NH], f32)
        nc.gpsimd.memset(ones_f[:], 1.0)
        nc.vector.tensor_copy(
            v_aug[:, :, :, DK],
            ones_f[:].rearrange("p (a b) -> p a b", a=SCK))
        for m in range(2):
            nc.gpsimd.memset(masks[m][:], 1.0)
            if causal:
                nc.gpsimd.affine_select(
                    masks[m][:], masks[m][:], pattern=[[-P, G], [1, SB]],
                    compare_op=is_ge, fill=0.0, base=-P * (G * m),
                    channel_multiplier=-1)
        nc.gpsimd.memset(mask_h[:], 1.0)
        if causal:
            nc.gpsimd.affine_select(
                mask_h[:], mask_h[:], pattern=[[-P, G], [1, HB]],
                compare_op=is_ge, fill=0.0, base=0,
                channel_multiplier=-1)

        bv_bc_r = bv_bc[:].rearrange("p (h e) -> p h e", h=NH)

        def proj_qk(sbi, xt):
            ss = slice(sbi * SB, (sbi + 1) * SB)
            # hc-major: head 0's q and k biases complete first, so its
            # score matmuls can start earliest
            for hc in range(HC):
                for name, w_sb, b_sb, t_sb in (
                        ("q", wq_sb, bq_sb, q_sb), ("k", wk_sb, bk_sb, k_sb)):
                    x = xt[name]
                    ps = proj_ps.tile([P, SB], f32, name="proj", tag="proj")
                    if qk_fp8:
                        # fp8 DoubleRow: two 128-deep k-subtiles per matmul
                        for j in range(DC // 2):
                            nc.tensor.matmul(
                                ps[:],
                                w_sb[:, 2 * j:2 * j + 2, hc * P:(hc + 1) * P],
                                x[:, 2 * j:2 * j + 2, :], perf_mode=DR,
                                start=(j == 0), stop=(j == DC // 2 - 1))
                    else:
                        for dc in range(DC):
                            nc.tensor.matmul(
                                ps[:], w_sb[:, dc, hc * P:(hc + 1) * P],
                                x[:, dc, :], start=(dc == 0),
                                stop=(dc == DC - 1))
                    if hc == 0 and sbi > 0:
                        # the first head-chunk's biases gate the next
                        # attention block's score matmuls; the scalar
                        # engine is idle at block boundaries while the
                        # vector engine still drains the previous tail
                        nc.scalar.activation(
                            t_sb[:, hc, ss], ps[:], Identity,
                            bias=b_sb[:, hc:hc + 1], scale=1.0)
                    else:
                        nc.vector.tensor_scalar_add(
                            t_sb[:, hc, ss], ps[:], b_sb[:, hc:hc + 1])

        def vproj_fillers(sbi, xt):
            """One filler per 128-seq chunk of the v projection."""
            x = xt["v"]

            def mk(scl):
                def run():
                    sc = sbi * (SB // P) + scl
                    ps = proj_ps.tile([P, DHL], f32, name="proj", tag="proj")
                    for dc in range(DC):
                        nc.tensor.matmul(
                            ps[:], x[:, dc, scl * P:(scl + 1) * P],
                            wv_sb[:, dc, :], start=(dc == 0),
                            stop=(dc == DC - 1))
                    nc.vector.tensor_add(
                        v_aug[:, sc, :, 0:DK],
                        ps[:].rearrange("p (h e) -> p h e", h=NH), bv_bc_r)
                return run
            return [mk(scl) for scl in range(SB // P)]

        def outproj_fillers(sbi, last=False):
            """One filler per output-dim chunk of the out projection."""
            ss = slice(sbi * SB, (sbi + 1) * SB)
            o_sb = opool.tile([P, DC, SB], mmdt, name=f"o{sbi}", tag="ot")

            def mk(oc):
                def run():
                    ps = proj_ps.tile([P, SB], f32, name="proj", tag="proj")
                    for hc in range(HC):
                        nc.tensor.matmul(
                            ps[:], wo_sb[:, hc, oc * P:(oc + 1) * P],
                            attn_sb[:, hc, ss], start=(hc == 0),
                            stop=(hc == HC - 1))
                    # psum -> sbuf bf16 staging; vector during attention
                    # (scalar is saturated by exp), alternated on the tail
                    if last and oc % 2 == 0:
                        nc.scalar.copy(o_sb[:, oc, :], ps[:])
                    else:
                        nc.vector.tensor_copy(o_sb[:, oc, :], ps[:])
                    # ship each half as soon as its copies land; on the
                    # final block, quarter granularity shortens the tail
                    if last:
                        if oc % 2 == 1:
                            nc.sync.dma_start(out_r[:, sbi, oc - 1:oc + 1],
                                              o_sb[:, oc - 1:oc + 1])
                    elif oc == DC // 2 - 1:
                        nc.sync.dma_start(out_r[:, sbi, 0:DC // 2],
                                          o_sb[:, 0:DC // 2])
                    elif oc == DC - 1:
                        nc.sync.dma_start(out_r[:, sbi, DC // 2:DC],
                                          o_sb[:, DC // 2:DC])
                return run
            return [mk(oc) for oc in range(DC)]

        def attn(qb, v_fill, fillers):
            """Attention for query block qb.  v_fill are the v-projection
            fillers for THIS block (issued inside head 0, just before its
            diagonal PV, so the scalar engine gets head 0's scores to exp
            during the q/k projections).  fillers (previous block's out
            projection) are issued as two coarse lumps inside heads 1 and
            2 at the same point, keeping the scalar engine fed across the
            whole block while preserving long same-kind matmul runs."""
            n_chunks = (qb + 1) * (SB // P) if causal else SCK
            n_groups = n_chunks // G
            lumps = {0: list(v_fill),
                     1: list(fillers[:len(fillers) // 2]),
                     2: list(fillers[len(fillers) // 2:])}

            for hl in range(NH):
                hc = hl // HPC
                po = (hl % HPC) * DK
                qs0 = qb * SB
                q_slice = q_sb[po:po + DK, hc, qs0:qs0 + SB]
                pv = pv_ps_pool.tile([DK + 1, SB], f32, name="pv", tag="pv")

                def issue_sc(g):
                    sc_t = sc_ps_pool.tile([P, G, SB], f32, name="sc",
                                           tag="sc")
                    for j2 in range(G):
                        tj = g * G + j2
                        nc.tensor.matmul(
                            sc_t[:, j2, :],
                            k_sb[po:po + DK, hc, tj * P:(tj + 1) * P],
                            q_slice, start=True, stop=True)
                    return sc_t

                if not causal:
                    sc_cur = issue_sc(0)
                    for g in range(n_groups):
                        sc_next = issue_sc(g + 1) if g + 1 < n_groups else None
                        ex = exp_pool.tile([P, G, SB], mmdt, name="ex",
                                           tag="ex")
                        nc.scalar.activation(ex[:], sc_cur[:], Exp, bias=0.0,
                                             scale=inv_sqrt_dk)
                        if g == n_groups - 2 and hl in lumps:
                            for f in lumps.pop(hl):
                                f()
                        for j2 in range(G):
                            tj = g * G + j2
                            nc.tensor.matmul(
                                pv[:], v_aug[:, tj, hl, :], ex[:, j2, :],
                                start=(g == 0 and j2 == 0),
                                stop=(g == n_groups - 1 and j2 == G - 1))
                        sc_cur = sc_next
                else:
                    # non-diagonal groups: full 512-query tiles, no mask.
                    # diagonal 4-chunk block: split into query halves —
                    # the first 256 queries only attend to the first two
                    # diagonal chunks, saving 25% of the diagonal scores,
                    # exp and PV work.
                    nd = n_groups - 2
                    c0 = n_chunks - 4
                    sc_cur = issue_sc(0) if nd > 0 else None
                    for g in range(nd):
                        # software pipeline: next group's score matmuls are
                        # issued before this group's PV so the PE never
                        # waits on exp
                        sc_next = issue_sc(g + 1) if g + 1 < nd else None
                        ex = exp_pool.tile([P, G, SB], mmdt, name="ex",
                                           tag="ex")
                        nc.scalar.activation(ex[:], sc_cur[:], Exp, bias=0.0,
                                             scale=inv_sqrt_dk)
                        for j2 in range(G):
                            tj = g * G + j2
                            nc.tensor.matmul(
                                pv[:], v_aug[:, tj, hl, :], ex[:, j2, :],
                                start=(g == 0 and j2 == 0), stop=False,
                                skip_group_check=True)
                        sc_cur = sc_next
                    # qsub0: queries [0,256), chunks c0..c0+1 (triangular)
                    sc0 = sc_ps_pool.tile([P, G, HB], f32, name="sc0",
                                          tag="sc")
                    for j2 in range(G):
                        nc.tensor.matmul(
                            sc0[:, j2, :],
                            k_sb[po:po + DK, hc,
                                 (c0 + j2) * P:(c0 + j2 + 1) * P],
                            q_sb[po:po + DK, hc, qs0:qs0 + HB],
                            start=True, stop=True)
                    # qsub1: queries [256,512), chunks c0..c0+3 (last two
                    # triangular)
                    sc1 = sc_ps_pool.tile([P, 4, HB], f32, name="sc1",
                                          tag="sc")
                    for j2 in range(4):
                        nc.tensor.matmul(
                            sc1[:, j2, :],
                            k_sb[po:po + DK, hc,
                                 (c0 + j2) * P:(c0 + j2 + 1) * P],
                            q_sb[po:po + DK, hc, qs0 + HB:qs0 + SB],
                            start=True, stop=True)
                    ex0 = exp_pool.tile([P, G, HB], mmdt, name="ex0",
                                        tag="ex")
                    nc.scalar.activation(ex0[:], sc0[:], Exp, bias=0.0,
                                         scale=inv_sqrt_dk)
                    nc.vector.tensor_mul(ex0[:], ex0[:], mask_h[:])
                    ex1 = exp_pool.tile([P, 4, HB], mmdt, name="ex1",
                                        tag="ex")
                    nc.scalar.activation(ex1[:], sc1[:], Exp, bias=0.0,
                                         scale=inv_sqrt_dk)
                    nc.vector.tensor_mul(ex1[:, 2:4, :], ex1[:, 2:4, :],
                                         mask_h[:])
                    if hl in lumps:
                        for f in lumps.pop(hl):
                            f()
                    for j2 in range(G):
                        nc.tensor.matmul(
                            pv[0:DK + 1, 0:HB],
                            v_aug[:, c0 + j2, hl, :], ex0[:, j2, :],
                            start=(nd == 0 and j2 == 0), stop=(j2 == G - 1),
                            skip_group_check=True)
                    for j2 in range(4):
                        nc.tensor.matmul(
                            pv[0:DK + 1, HB:SB],
                            v_aug[:, c0 + j2, hl, :], ex1[:, j2, :],
                            start=(nd == 0 and j2 == 0), stop=(j2 == 3),
                            skip_group_check=True)
                den = small.tile([1, SB], f32, name="den", tag="den")
                nc.vector.tensor_copy(den[:], pv[DK:DK + 1, :])
                recip = small.tile([1, SB], f32, name="recip", tag="recip")
                nc.vector.reciprocal_approx_fast(recip[:], den[:])
                recip_bc = small.tile([DK, SB], f32, name="recip_bc",
                                      tag="rbc")
                nc.gpsimd.partition_broadcast(recip_bc[:], recip[:])
                nc.vector.tensor_mul(
                    attn_sb[po:po + DK, hc, qb * SB:(qb + 1) * SB],
                    pv[0:DK, :], recip_bc[:])
            for lump in lumps.values():
                for f in lump:
                    f()

        if causal:
            # --- interleaved schedule: per query block, q/k projections are
            # issued first, then attention with the v projection of this
            # block and the out projection of the previous block woven in
            # as fillers
            proj_qk(0, xt)
            vf = vproj_fillers(0, xt)
            xt = load_x(1)
            attn(0, vf, [])
            for sbi in range(1, NSB):
                proj_qk(sbi, xt)
                vf = vproj_fillers(sbi, xt)
                xt = load_x(sbi + 1) if sbi + 1 < NSB else None
                attn(sbi, vf, outproj_fillers(sbi - 1))
            for f in outproj_fillers(NSB - 1, last=True):
                f()
        else:
            # dense attention reads every key block, so all projections
            # must be issued before any attention
            for sbi in range(NSB):
                proj_qk(sbi, xt)
                for f in vproj_fillers(sbi, xt):
                    f()
                xt = load_x(sbi + 1) if sbi + 1 < NSB else None
            for qb in range(NSB):
                attn(qb, [], outproj_fillers(qb - 1) if qb else [])
            for f in outproj_fillers(NSB - 1, last=True):
                f()

    nc.compile()
    return nc


def _get_nc(causal):
    key = ("causal" if causal else "dense", QK_FP8)
    if key not in _CACHE:
        _CACHE[key] = _build_nc(causal, QK_FP8)
    return _CACHE[key]


def _arrange_x(x, dt_):
    """[S, D] f32 -> [P, NSB*DC*SB] with layout [p, sbi, dc, s']."""
    a = x.T.reshape(DC, P, NSB, SB).transpose(1, 2, 0, 3)
    return np.ascontiguousarray(a.reshape(P, NSB * DC * SB)).astype(dt_)


def _prep_core_inputs(Q, K, V, Wq, bq, Wk, bk, Wv, bv, Wo):
    """Build the 8 per-core input maps (all arrays C-contiguous)."""
    import ml_dtypes
    bf16 = ml_dtypes.bfloat16
    qk_dt = ml_dtypes.float8_e4m3 if QK_FP8 else bf16
    cc = np.ascontiguousarray
    x_arr = {}
    for b in range(B):
        x_arr[("q", b)] = _arrange_x(Q[b], qk_dt)
        x_arr[("k", b)] = _arrange_x(K[b], qk_dt)
        x_arr[("v", b)] = _arrange_x(V[b], bf16)
    in_maps = []
    for c in range(NCORES):
        b = c // NGROUPS
        g = c % NGROUPS
        hs, he = g * DHL, (g + 1) * DHL
        # weights pre-arranged to SBUF layout [128, DC, DHL] with d = dc*128+p
        wq_a = cc(Wq[hs:he, :].T.reshape(DC, P, DHL).transpose(1, 0, 2)
                  .reshape(P, DC * DHL))
        wk_a = cc(Wk[hs:he, :].T.reshape(DC, P, DHL).transpose(1, 0, 2)
                  .reshape(P, DC * DHL))
        wv_a = cc(Wv[hs:he, :].T.reshape(DC, P, DHL).transpose(1, 0, 2)
                  .reshape(P, DC * DHL))
        # Wo shard: lhsT layout [hd, dout] split to [128, HC, D], hd = hc*128+p
        wo_a = cc(Wo[:, hs:he].T.reshape(HC, P, D).transpose(1, 0, 2)
                  .reshape(P, HC * D))
        in_maps.append({
            "xq_a": x_arr[("q", b)], "xk_a": x_arr[("k", b)],
            "xv_a": x_arr[("v", b)],
            "wq_a": wq_a.astype(qk_dt), "wk_a": wk_a.astype(qk_dt),
            "wv_a": wv_a.astype(bf16), "wo_a": wo_a.astype(bf16),
            "bq_a": cc(bq[hs:he].reshape(HC, P).T),
            "bk_a": cc(bk[hs:he].reshape(HC, P).T),
            "bv_a": cc(bv[hs:he].reshape(1, DHL)),
        })
    return in_maps


def _classify_mask(mask):
    m = np.asarray(mask)
    if m.dtype != np.bool_:
        m = m.astype(bool)
    causal = np.tril(np.ones((S, S), dtype=bool))
    if all(np.array_equal(m[b, 0], causal) for b in range(m.shape[0])):
        return "causal"
    if m.all():
        return "dense"
    return "generic"


def _numpy_reference(Q, K, V, mask, Wq, bq, Wk, bk, Wv, bv, Wo, bo):
    """Float64-free plain numpy fallback for arbitrary masks."""
    out = np.empty((B, S, D), dtype=np.float32)
    for b in range(B):
        q = (Q[b] @ Wq.T + bq).reshape(S, H, DK).transpose(1, 0, 2)
        k = (K[b] @ Wk.T + bk).reshape(S, H, DK).transpose(1, 0, 2)
        v = (V[b] @ Wv.T + bv).reshape(S, H, DK).transpose(1, 0, 2)
        m = np.asarray(mask[b, 0], dtype=bool)
        acc = np.empty((H, S, DK), dtype=np.float32)
        for h in range(H):
            s = (q[h] @ k[h].T) / np.float32(np.sqrt(DK))
            s = np.where(m, s, np.float32(-1e9))
            s = s - s.max(axis=-1, keepdims=True)
            e = np.exp(s)
            p = e / e.sum(axis=-1, keepdims=True)
            acc[h] = p @ v[h]
        out[b] = acc.transpose(1, 0, 2).reshape(S, D) @ Wo.T + bo
    return out


def _unarrange_out(a):
    """[P, NSB*DC*SB] -> [S, D] f32."""
    t = a.reshape(P, NSB, DC, SB).astype(np.float32)
    return t.transpose(1, 3, 2, 0).reshape(S, D)


def kernel(Q, K, V, mask, Wq, bq, Wk, bk, Wv, bv, Wo, bo,
           _profile=False, _trace_dir=None):
    from concourse.bass_utils import run_bass_kernel_spmd

    flavor = _classify_mask(mask)
    if flavor == "generic":
        return _numpy_reference(Q, K, V, mask, Wq, bq, Wk, bk, Wv, bv, Wo, bo)

    nc = _get_nc(flavor == "causal")
    in_maps = _prep_core_inputs(
        np.asarray(Q, np.float32), np.asarray(K, np.float32),
        np.asarray(V, np.float32), np.asarray(Wq, np.float32),
        np.asarray(bq, np.float32), np.asarray(Wk, np.float32),
        np.asarray(bk, np.float32), np.asarray(Wv, np.float32),
        np.asarray(bv, np.float32), np.asarray(Wo, np.float32))

    kwargs = {}
    if _profile:
        import types
        if "antenv.axon_hooks" not in sys.modules:
            _mod = types.ModuleType("antenv.axon_hooks")
            _mod._hook = None
            _mod.set_axon_ntff_profile_hook = (
                lambda h, _m=_mod: setattr(_m, "_hook", h))
            _mod.get_axon_ntff_profile_hook = lambda _m=_mod: _m._hook
            sys.modules["antenv.axon_hooks"] = _mod
            try:
                import antenv
                antenv.axon_hooks = _mod
            except ImportError:
                pass
        _mod = sys.modules["antenv.axon_hooks"]
        if _mod.get_axon_ntff_profile_hook() is None:
            from trn_agent_boot.trn_boot import _ntff_profile_via_ctypes
            _mod.set_axon_ntff_profile_hook(
                _ntff_profile_via_ctypes("/opt/axon/libaxon_pjrt.so"))
        import concourse.bass_utils as _bu
        _bu.upload_artifacts = lambda d: d  # no cloud copy in this container
        kwargs = dict(trace=True, trace_cores=[0])
        if _trace_dir is not None:
            kwargs["tmpdir"] = _trace_dir
    res = run_bass_kernel_spmd(nc, in_maps, core_ids=list(range(NCORES)),
                               **kwargs)

    out = np.empty((B, S, D), dtype=np.float32)
    bo32 = np.asarray(bo, np.float32)
    for b in range(B):
        acc = _unarrange_out(np.asarray(res.results[b * NGROUPS]["out_a"]))
        for g in range(1, NGROUPS):
            acc = acc + _unarrange_out(
                np.asarray(res.results[b * NGROUPS + g]["out_a"]))
        out[b] = acc + bo32
    if _profile:
        kernel._last_exec_time_ns = res.exec_time_ns
        kernel._last_results = res
    return out


# revision 34
# speedup vs baseline: 1.0524x; 1.0036x over previous
"""Multi-head causal attention (B=2, S=2048, D=1024, H=16) on 8 Trainium2
NeuronCores.

Sharding: data-parallel over the 2 batches x tensor-parallel over 4 head
groups (4 heads each).  Core c handles batch c//4, heads [4*(c%4), 4*(c%4)+4).
Each core computes its Q/K/V projections from column shards of Wq/Wk/Wv,
runs causal attention for its heads, and applies its row shard of Wo,
producing a partial (D, S) output in bf16.  The host sums the 4 partials
per batch in f32 and adds the output bias.

On-core layout: activations are kept transposed (feature dim on SBUF
partitions, sequence on the free axis) so every matmul's operands are
already in the (K x M)/(K x N) form the PE array wants, and the softmax
denominator comes free from an extra ones-row appended to V.

Schedule: phases are interleaved per 512-query block
(proj(0), attn(0), proj(1), outproj(0), attn(1), ...) and the attention
inner loop issues score matmuls one group ahead of the PV matmuls, so the
tensor engine never waits on the scalar-engine exp and stays continuously
busy (keeps the PE at its top p-state clock).  The causal mask is applied
by multiplying the exp'd scores with precomputed bf16 0/1 tiles on the
vector engine; softmax division uses the fast approximate reciprocal.
"""

import sys

sys.path.insert(0, "/opt/trn_rl_repo")

import numpy as np

B, S, D, H = 2, 2048, 1024, 16
DK = D // H            # 64 head dim
NCORES = 8
NGROUPS = 4            # head groups (tensor parallel)
NH = H // NGROUPS      # 4 heads per core
DHL = NH * DK          # 256 local head dims per core
P = 128
DC = D // P            # 8 contraction chunks over D
HC = DHL // P          # 2 local head-dim chunks
SB = 512               # query block (matmul moving free size)
NSB = S // SB          # 4
SCK = S // P           # 16 key chunks
HPC = P // DK          # 2 heads per head-dim chunk
G = 2                  # score chunks per exp group (2 PSUM banks)

# fp8(e4m3) DoubleRow matmuls for the Q/K projections (V and the output
# projection stay bf16 — fp8 there fails the 2e-2 accuracy gate).
QK_FP8 = True

_CACHE = {}


def _build_nc(causal, qk_fp8):
    import concourse.bass as bass
    import concourse.bacc as bacc
    import concourse.mybir as mybir
    import concourse.tile as tile
    from contextlib import ExitStack

    f32 = mybir.dt.float32
    mmdt = mybir.dt.bfloat16
    qkdt = mybir.dt.float8e4 if qk_fp8 else mmdt
    DR = mybir.MatmulPerfMode.DoubleRow if qk_fp8 else None
    Exp = mybir.ActivationFunctionType.Exp
    Identity = mybir.ActivationFunctionType.Identity
    is_ge = mybir.AluOpType.is_ge

    nc = bacc.Bacc(None, target_bir_lowering=False, debug=False)

    # x pre-arranged on host to [P, NSB, DC, SB]: per-partition contiguous
    # 8KB blocks per sequence block -> clean 2D DMA.
    xq_d = nc.dram_tensor("xq_a", [P, NSB * DC * SB], qkdt, kind="ExternalInput")
    xk_d = nc.dram_tensor("xk_a", [P, NSB * DC * SB], qkdt, kind="ExternalInput")
    xv_d = nc.dram_tensor("xv_a", [P, NSB * DC * SB], mmdt, kind="ExternalInput")
    # weights pre-arranged on host to the exact SBUF layouts
    wq_d = nc.dram_tensor("wq_a", [P, DC * DHL], qkdt, kind="ExternalInput")
    wk_d = nc.dram_tensor("wk_a", [P, DC * DHL], qkdt, kind="ExternalInput")
    wv_d = nc.dram_tensor("wv_a", [P, DC * DHL], mmdt, kind="ExternalInput")
    wo_d = nc.dram_tensor("wo_a", [P, HC * D], mmdt, kind="ExternalInput")
    bq_d = nc.dram_tensor("bq_a", [P, HC], f32, kind="ExternalInput")
    bk_d = nc.dram_tensor("bk_a", [P, HC], f32, kind="ExternalInput")
    bv_d = nc.dram_tensor("bv_a", [1, DHL], f32, kind="ExternalInput")
    # bf16 partial output, same [P, NSB, DC, SB] arrangement
    out_d = nc.dram_tensor("out_a", [P, NSB * DC * SB], mmdt,
                           kind="ExternalOutput")

    xq_r = xq_d[:].rearrange("p (b c s) -> p b c s", b=NSB, c=DC)
    xk_r = xk_d[:].rearrange("p (b c s) -> p b c s", b=NSB, c=DC)
    xv_r = xv_d[:].rearrange("p (b c s) -> p b c s", b=NSB, c=DC)
    out_r = out_d[:].rearrange("p (b c s) -> p b c s", b=NSB, c=DC)

    inv_sqrt_dk = 1.0 / float(np.sqrt(DK))

    with tile.TileContext(nc) as tc, ExitStack() as ctx:
        consts = ctx.enter_context(tc.tile_pool(name="consts", bufs=1))
        xpool = ctx.enter_context(tc.tile_pool(name="xpool", bufs=6))
        exp_pool = ctx.enter_context(tc.tile_pool(name="exp_pool", bufs=3))
        small = ctx.enter_context(tc.tile_pool(name="small", bufs=6))
        opool = ctx.enter_context(tc.tile_pool(name="opool", bufs=2))
        proj_ps = ctx.enter_context(
            tc.tile_pool(name="proj_ps", bufs=2, space="PSUM"))
        sc_ps_pool = ctx.enter_context(
            tc.tile_pool(name="sc_ps", bufs=2, space="PSUM"))
        pv_ps_pool = ctx.enter_context(
            tc.tile_pool(name="pv_ps", bufs=2, space="PSUM"))

        # --- resident tensors ---
        wq_sb = consts.tile([P, DC, DHL], qkdt)
        wk_sb = consts.tile([P, DC, DHL], qkdt)
        wv_sb = consts.tile([P, DC, DHL], mmdt)
        wo_sb = consts.tile([P, HC, D], mmdt)
        bq_sb = consts.tile([P, HC], f32)
        bk_sb = consts.tile([P, HC], f32)
        bv_row = consts.tile([1, DHL], f32)
        bv_bc = consts.tile([P, DHL], f32)
        q_sb = consts.tile([P, HC, S], mmdt)
        k_sb = consts.tile([P, HC, S], mmdt)
        v_aug = consts.tile([P, SCK, NH, DK + 1], mmdt)
        attn_sb = consts.tile([P, HC, S], mmdt)
        # causal 0/1 masks for the two diagonal groups: plane j2 of mask
        # tile m keeps (p, i) when i - p - 128*(2*m + j2) >= 0
        masks = [consts.tile([P, G, SB], mmdt, name=f"mask{m}")
                 for m in range(2)]
        # half-block mask for the split diagonal: keeps (p, j2, i') when
        # i' - p - 128*j2 >= 0 over a 256-query half block
        HB = SB // 2
        mask_h = consts.tile([P, G, HB], mmdt, name="mask_h")

        def load_x(sbi):
            tiles = {}
            for name, src, dt_ in (("q", xq_r, qkdt), ("k", xk_r, qkdt),
                                   ("v", xv_r, mmdt)):
                t = xpool.tile([P, DC, SB], dt_, name=f"x{name}{sbi}",
                               tag="xs")
                # q/k ride the gpsimd SWDGE queue (idle in steady state)
                # so they never queue behind the sync-queue transfers
                eng = nc.sync if name == "v" else nc.gpsimd
                eng.dma_start(t[:], src[:, sbi])
                tiles[name] = t
            return tiles

        # --- startup: weight DMAs split across the two HWDGE queues
        # (sync + scalar) with the first wq / xq chunk pair shipped first,
        # so the first projection matmul can start as early as possible ---
        wq_r = wq_d[:].rearrange("p (c h) -> p c h", c=DC)
        wk_r = wk_d[:].rearrange("p (c h) -> p c h", c=DC)
        xq_t = xpool.tile([P, DC, SB], qkdt, name="xq0", tag="xs")
        xk_t = xpool.tile([P, DC, SB], qkdt, name="xk0", tag="xs")
        xv_t = xpool.tile([P, DC, SB], mmdt, name="xv0", tag="xs")
        # pair-granular first-block loads pipelined over three DMA queues,
        # in exactly the order the first q/k projections consume them
        for j in range(0, DC, 2):
            nc.sync.dma_start(wq_sb[:, j:j + 2], wq_r[:, j:j + 2])
            nc.scalar.dma_start(xq_t[:, j:j + 2], xq_r[:, 0, j:j + 2])
            nc.gpsimd.dma_start(xk_t[:, j:j + 2], xk_r[:, 0, j:j + 2])
        nc.scalar.dma_start(bq_sb[:], bq_d[:])
        for j in range(0, DC, 2):
            nc.sync.dma_start(wk_sb[:, j:j + 2], wk_r[:, j:j + 2])
        nc.scalar.dma_start(bk_sb[:], bk_d[:])
        nc.gpsimd.dma_start(xv_t[:], xv_r[:, 0])
        nc.sync.dma_start(wv_sb[:], wv_d[:].rearrange("p (c h) -> p c h",
                                                      c=DC))
        nc.scalar.dma_start(bv_row[:], bv_d[:])
        nc.scalar.dma_start(wo_sb[:], wo_d[:].rearrange("p (c o) -> p c o",
                                                        c=HC))
        xt = {"q": xq_t, "k": xk_t, "v": xv_t}

        nc.gpsimd.partition_broadcast(bv_bc[:], bv_row[:])
        ones_f = consts.tile([P, SCK * NH], f32)
        nc.gpsimd.memset(ones_f[:], 1.0)
        nc.vector.tensor_copy(
            v_aug[:, :, :, DK],
            ones_f[:].rearrange("p (a b) -> p a b", a=SCK))
        for m in range(2):
            nc.gpsimd.memset(masks[m][:], 1.0)
            if causal:
                nc.gpsimd.affine_select(
                    masks[m][:], masks[m][:], pattern=[[-P, G], [1, SB]],
                    compare_op=is_ge, fill=0.0, base=-P * (G * m),
                    channel_multiplier=-1)
        nc.gpsimd.memset(mask_h[:], 1.0)
        if causal:
            nc.gpsimd.affine_select(
                mask_h[:], mask_h[:], pattern=[[-P, G], [1, HB]],
                compare_op=is_ge, fill=0.0, base=0,
                channel_multiplier=-1)

        bv_bc_r = bv_bc[:].rearrange("p (h e) -> p h e", h=NH)

        def proj_qk(sbi, xt):
            ss = slice(sbi * SB, (sbi + 1) * SB)
            # hc-major: head 0's q and k biases complete first, so its
            # score matmuls can start earliest
            for hc in range(HC):
                for name, w_sb, b_sb, t_sb in (
                        ("q", wq_sb, bq_sb, q_sb), ("k", wk_sb, bk_sb, k_sb)):
                    x = xt[name]
                    ps = proj_ps.tile([P, SB], f32, name="proj", tag="proj")
                    if qk_fp8:
                        # fp8 DoubleRow: two 128-deep k-subtiles per matmul
                        for j in range(DC // 2):
                            nc.tensor.matmul(
                                ps[:],
                                w_sb[:, 2 * j:2 * j + 2, hc * P:(hc + 1) * P],
                                x[:, 2 * j:2 * j + 2, :], perf_mode=DR,
                                start=(j == 0), stop=(j == DC // 2 - 1))
                    else:
                        for dc in range(DC):
                            nc.tensor.matmul(
                                ps[:], w_sb[:, dc, hc * P:(hc + 1) * P],
                                x[:, dc, :], start=(dc == 0),
                                stop=(dc == DC - 1))
                    if hc == 0 and sbi > 0:
                        # the first head-chunk's biases gate the next
                        # attention block's score matmuls; the scalar
                        # engine is idle at block boundaries while the
                        # vector engine still drains the previous tail
                        nc.scalar.activation(
                            t_sb[:, hc, ss], ps[:], Identity,
                            bias=b_sb[:, hc:hc + 1], scale=1.0)
                    else:
                        nc.vector.tensor_scalar_add(
                            t_sb[:, hc, ss], ps[:], b_sb[:, hc:hc + 1])

        def vproj_fillers(sbi, xt):
            """One filler per 128-seq chunk of the v projection."""
            x = xt["v"]

            def mk(scl):
                def run():
                    sc = sbi * (SB // P) + scl
                    ps = proj_ps.tile([P, DHL], f32, name="proj", tag="proj")
                    for dc in range(DC):
                        nc.tensor.matmul(
                            ps[:], x[:, dc, scl * P:(scl + 1) * P],
                            wv_sb[:, dc, :], start=(dc == 0),
                            stop=(dc == DC - 1))
                    nc.vector.tensor_add(
                        v_aug[:, sc, :, 0:DK],
                        ps[:].rearrange("p (h e) -> p h e", h=NH), bv_bc_r)
                return run
            return [mk(scl) for scl in range(SB // P)]

        def outproj_fillers(sbi, last=False):
            """One filler per output-dim chunk of the out projection."""
            ss = slice(sbi * SB, (sbi + 1) * SB)
            o_sb = opool.tile([P, DC, SB], mmdt, name=f"o{sbi}", tag="ot")

            def mk(oc):
                def run():
                    ps = proj_ps.tile([P, SB], f32, name="proj", tag="proj")
                    for hc in range(HC):
                        nc.tensor.matmul(
                            ps[:], wo_sb[:, hc, oc * P:(oc + 1) * P],
                            attn_sb[:, hc, ss], start=(hc == 0),
                            stop=(hc == HC - 1))
                    # psum -> sbuf bf16 staging; vector during attention
                    # (scalar is saturated by exp), alternated on the tail
                    if last and oc % 2 == 0:
                        nc.scalar.copy(o_sb[:, oc, :], ps[:])
                    else:
                        nc.vector.tensor_copy(o_sb[:, oc, :], ps[:])
                    # ship each half as soon as its copies land; on the
                    # final block, quarter granularity shortens the tail
                    if last:
                        if oc % 2 == 1:
                            nc.sync.dma_start(out_r[:, sbi, oc - 1:oc + 1],
                                              o_sb[:, oc - 1:oc + 1])
                    elif oc == DC // 2 - 1:
                        nc.sync.dma_start(out_r[:, sbi, 0:DC // 2],
                                          o_sb[:, 0:DC // 2])
                    elif oc == DC - 1:
                        nc.sync.dma_start(out_r[:, sbi, DC // 2:DC],
                                          o_sb[:, DC // 2:DC])
                return run
            return [mk(oc) for oc in range(DC)]

        def attn(qb, v_fill, fillers):
            """Attention for query block qb.  v_fill are the v-projection
            fillers for THIS block (issued inside head 0, just before its
            diagonal PV, so the scalar engine gets head 0's scores to exp
            during the q/k projections).  fillers (previous block's out
            projection) are issued as two coarse lumps inside heads 1 and
            2 at the same point, keeping the scalar engine fed across the
            whole block while preserving long same-kind matmul runs."""
            n_chunks = (qb + 1) * (SB // P) if causal else SCK
            n_groups = n_chunks // G
            lumps = {0: list(v_fill),
                     1: list(fillers[:len(fillers) // 2]),
                     2: list(fillers[len(fillers) // 2:])}

            for hl in range(NH):
                hc = hl // HPC
                po = (hl % HPC) * DK
                qs0 = qb * SB
                q_slice = q_sb[po:po + DK, hc, qs0:qs0 + SB]
                pv = pv_ps_pool.tile([DK + 1, SB], f32, name="pv", tag="pv")

                def issue_sc(g):
                    sc_t = sc_ps_pool.tile([P, G, SB], f32, name="sc",
                                           tag="sc")
                    for j2 in range(G):
                        tj = g * G + j2
                        nc.tensor.matmul(
                            sc_t[:, j2, :],
                            k_sb[po:po + DK, hc, tj * P:(tj + 1) * P],
                            q_slice, start=True, stop=True)
                    return sc_t

                if not causal:
                    sc_cur = issue_sc(0)
                    for g in range(n_groups):
                        sc_next = issue_sc(g + 1) if g + 1 < n_groups else None
                        ex = exp_pool.tile([P, G, SB], mmdt, name="ex",
                                           tag="ex")
                        nc.scalar.activation(ex[:], sc_cur[:], Exp, bias=0.0,
                                             scale=inv_sqrt_dk)
                        if g == n_groups - 2 and hl in lumps:
                            for f in lumps.pop(hl):
                                f()
                        for j2 in range(G):
                            tj = g * G + j2
                            nc.tensor.matmul(
                                pv[:], v_aug[:, tj, hl, :], ex[:, j2, :],
                                start=(g == 0 and j2 == 0),
                                stop=(g == n_groups - 1 and j2 == G - 1))
                        sc_cur = sc_next
                else:
                    # non-diagonal groups: full 512-query tiles, no mask.
                    # diagonal 4-chunk block: split into query halves —
                    # the first 256 queries only attend to the first two
                    # diagonal chunks, saving 25% of the diagonal scores,
                    # exp and PV work.
                    nd = n_groups - 2
                    c0 = n_chunks - 4
                    sc_cur = issue_sc(0) if nd > 0 else None
                    for g in range(nd):
                        # software pipeline: next group's score matmuls are
                        # issued before this group's PV so the PE never
                        # waits on exp
                        sc_next = issue_sc(g + 1) if g + 1 < nd else None
                        ex = exp_pool.tile([P, G, SB], mmdt, name="ex",
                                           tag="ex")
                        nc.scalar.activation(ex[:], sc_cur[:], Exp, bias=0.0,
                                             scale=inv_sqrt_dk)
                        for j2 in range(G):
                            tj = g * G + j2
                            nc.tensor.matmul(
                                pv[:], v_aug[:, tj, hl, :], ex[:, j2, :],
                                start=(g == 0 and j2 == 0), stop=False,
                                skip_group_check=True)
                        sc_cur = sc_next
                    # qsub0: queries [0,256), chunks c0..c0+1 (triangular)
                    sc0 = sc_ps_pool.tile([P, G, HB], f32, name="sc0",
                                          tag="sc")
                    for j2 in range(G):
                        nc.tensor.matmul(
                            sc0[:, j2, :],
                            k_sb[po:po + DK, hc,
                                 (c0 + j2) * P:(c0 + j2 + 1) * P],
                            q_sb[po:po + DK, hc, qs0:qs0 + HB],
                            start=True, stop=True)
                    # qsub1: queries [256,512), chunks c0..c0+3 (last two
                    # triangular)
                    sc1 = sc_ps_pool.tile([P, 4, HB], f32, name="sc1",
                                          tag="sc")
                    for j2 in range(4):
                        nc.tensor.matmul(
                            sc1[:, j2, :],
                            k_sb[po:po + DK, hc,
                                 (c0 + j2) * P:(c0 + j2 + 1) * P],
                            q_sb[po:po + DK, hc, qs0 + HB:qs0 + SB],
                            start=True, stop=True)
                    ex0 = exp_pool.tile([P, G, HB], mmdt, name="ex0",
                                        tag="ex")
                    nc.scalar.activation(ex0[:], sc0[:], Exp, bias=0.0,
                                         scale=inv_sqrt_dk)
                    nc.vector.tensor_mul(ex0[:], ex0[:], mask_h[:])
                    ex1 = exp_pool.tile([P, 4, HB], mmdt, name="ex1",
                                        tag="ex")
                    nc.scalar.activation(ex1[:], sc1[:], Exp, bias=0.0,
                                         scale=inv_sqrt_dk)
                    nc.vector.tensor_mul(ex1[:, 2:4, :], ex1[:, 2:4, :],
                                         mask_h[:])
                    if hl in lumps:
                        for f in lumps.pop(hl):
                            f()
                    for j2 in range(G):
                        nc.tensor.matmul(
                            pv[0:DK + 1, 0:HB],
                            v_aug[:, c0 + j2, hl, :], ex0[:, j2, :],
                            start=(nd == 0 and j2 == 0), stop=(j2 == G - 1),
                            skip_group_check=True)
                    for j2 in range(4):
                        nc.tensor.matmul(
                            pv[0:DK + 1, HB:SB],
                            v_aug[:, c0 + j2, hl, :], ex1[:, j2, :],
                            start=(nd == 0 and j2 == 0), stop=(j2 == 3),
                            skip_group_check=True)
                den = small.tile([1, SB], f32, name="den", tag="den")
                nc.vector.tensor_copy(den[:], pv[DK:DK + 1, :])
                recip = small.tile([1, SB], f32, name="recip", tag="recip")
                nc.vector.reciprocal_approx_fast(recip[:], den[:])
                recip_bc = small.tile([DK, SB], f32, name="recip_bc",
                                      tag="rbc")
                nc.gpsimd.partition_broadcast(recip_bc[:], recip[:])
                nc.vector.tensor_mul(
                    attn_sb[po:po + DK, hc, qb * SB:(qb + 1) * SB],
                    pv[0:DK, :], recip_bc[:])
            for lump in lumps.values():
                for f in lump:
                    f()

        if causal:
            # --- interleaved schedule: per query block, q/k projections are
            # issued first, then attention with the v projection of this
            # block and the out projection of the previous block woven in
            # as fillers
            proj_qk(0, xt)
            vf = vproj_fillers(0, xt)
            xt = load_x(1)
            attn(0, vf, [])
            for sbi in range(1, NSB):
                proj_qk(sbi, xt)
                vf = vproj_fillers(sbi, xt)
                xt = load_x(sbi + 1) if sbi + 1 < NSB else None
                attn(sbi, vf, outproj_fillers(sbi - 1))
            for f in outproj_fillers(NSB - 1, last=True):
                f()
        else:
            # dense attention reads every key block, so all projections
            # must be issued before any attention
            for sbi in range(NSB):
                proj_qk(sbi, xt)
                for f in vproj_fillers(sbi, xt):
                    f()
                xt = load_x(sbi + 1) if sbi + 1 < NSB else None
            for qb in range(NSB):
                attn(qb, [], outproj_fillers(qb - 1) if qb else [])
            for f in outproj_fillers(NSB - 1, last=True):
                f()

    nc.compile()
    return nc


def _get_nc(causal):
    key = ("causal" if causal else "dense", QK_FP8)
    if key not in _CACHE:
        _CACHE[key] = _build_nc(causal, QK_FP8)
    return _CACHE[key]


def _arrange_x(x, dt_):
    """[S, D] f32 -> [P, NSB*DC*SB] with layout [p, sbi, dc, s']."""
    a = x.T.reshape(DC, P, NSB, SB).transpose(1, 2, 0, 3)
    return np.ascontiguousarray(a.reshape(P, NSB * DC * SB)).astype(dt_)


def _prep_core_inputs(Q, K, V, Wq, bq, Wk, bk, Wv, bv, Wo):
    """Build the 8 per-core input maps (all arrays C-contiguous)."""
    import ml_dtypes
    bf16 = ml_dtypes.bfloat16
    qk_dt = ml_dtypes.float8_e4m3 if QK_FP8 else bf16
    cc = np.ascontiguousarray
    x_arr = {}
    for b in range(B):
        x_arr[("q", b)] = _arrange_x(Q[b], qk_dt)
        x_arr[("k", b)] = _arrange_x(K[b], qk_dt)
        x_arr[("v", b)] = _arrange_x(V[b], bf16)
    in_maps = []
    for c in range(NCORES):
        b = c // NGROUPS
        g = c % NGROUPS
        hs, he = g * DHL, (g + 1) * DHL
        # weights pre-arranged to SBUF layout [128, DC, DHL] with d = dc*128+p
        wq_a = cc(Wq[hs:he, :].T.reshape(DC, P, DHL).transpose(1, 0, 2)
                  .reshape(P, DC * DHL))
        wk_a = cc(Wk[hs:he, :].T.reshape(DC, P, DHL).transpose(1, 0, 2)
                  .reshape(P, DC * DHL))
        wv_a = cc(Wv[hs:he, :].T.reshape(DC, P, DHL).transpose(1, 0, 2)
                  .reshape(P, DC * DHL))
        # Wo shard: lhsT layout [hd, dout] split to [128, HC, D], hd = hc*128+p
        wo_a = cc(Wo[:, hs:he].T.reshape(HC, P, D).transpose(1, 0, 2)
                  .reshape(P, HC * D))
        in_maps.append({
            "xq_a": x_arr[("q", b)], "xk_a": x_arr[("k", b)],
            "xv_a": x_arr[("v", b)],
            "wq_a": wq_a.astype(qk_dt), "wk_a": wk_a.astype(qk_dt),
            "wv_a": wv_a.astype(bf16), "wo_a": wo_a.astype(bf16),
            "bq_a": cc(bq[hs:he].reshape(HC, P).T),
            "bk_a": cc(bk[hs:he].reshape(HC, P).T),
            "bv_a": cc(bv[hs:he].reshape(1, DHL)),
        })
    return in_maps


def _classify_mask(mask):
    m = np.asarray(mask)
    if m.dtype != np.bool_:
        m = m.astype(bool)
    causal = np.tril(np.ones((S, S), dtype=bool))
    if all(np.array_equal(m[b, 0], causal) for b in range(m.shape[0])):
        return "causal"
    if m.all():
        return "dense"
    return "generic"


def _numpy_reference(Q, K, V, mask, Wq, bq, Wk, bk, Wv, bv, Wo, bo):
    """Float64-free plain numpy fallback for arbitrary masks."""
    out = np.empty((B, S, D), dtype=np.float32)
    for b in range(B):
        q = (Q[b] @ Wq.T + bq).reshape(S, H, DK).transpose(1, 0, 2)
        k = (K[b] @ Wk.T + bk).reshape(S, H, DK).transpose(1, 0, 2)
        v = (V[b] @ Wv.T + bv).reshape(S, H, DK).transpose(1, 0, 2)
        m = np.asarray(mask[b, 0], dtype=bool)
        acc = np.empty((H, S, DK), dtype=np.float32)
        for h in range(H):
            s = (q[h] @ k[h].T) / np.float32(np.sqrt(DK))
            s = np.where(m, s, np.float32(-1e9))
            s = s - s.max(axis=-1, keepdims=True)
            e = np.exp(s)
            p = e / e.sum(axis=-1, keepdims=True)
            acc[h] = p @ v[h]
        out[b] = acc.transpose(1, 0, 2).reshape(S, D) @ Wo.T + bo
    return out


def _unarrange_out(a):
    """[P, NSB*DC*SB] -> [S, D] f32."""
    t = a.reshape(P, NSB, DC, SB).astype(np.float32)
    return t.transpose(1, 3, 2, 0).reshape(S, D)


def kernel(Q, K, V, mask, Wq, bq, Wk, bk, Wv, bv, Wo, bo,
           _profile=False, _trace_dir=None):
    from concourse.bass_utils import run_bass_kernel_spmd

    flavor = _classify_mask(mask)
    if flavor == "generic":
        return _numpy_reference(Q, K, V, mask, Wq, bq, Wk, bk, Wv, bv, Wo, bo)

    nc = _get_nc(flavor == "causal")
    in_maps = _prep_core_inputs(
        np.asarray(Q, np.float32), np.asarray(K, np.float32),
        np.asarray(V, np.float32), np.asarray(Wq, np.float32),
        np.asarray(bq, np.float32), np.asarray(Wk, np.float32),
        np.asarray(bk, np.float32), np.asarray(Wv, np.float32),
        np.asarray(bv, np.float32), np.asarray(Wo, np.float32))

    kwargs = {}
    if _profile:
        import types
        if "antenv.axon_hooks" not in sys.modules:
            _mod = types.ModuleType("antenv.axon_hooks")
            _mod._hook = None
            _mod.set_axon_ntff_profile_hook = (
                lambda h, _m=_mod: setattr(_m, "_hook", h))
            _mod.get_axon_ntff_profile_hook = lambda _m=_mod: _m._hook
            sys.modules["antenv.axon_hooks"] = _mod
            try:
                import antenv
                antenv.axon_hooks = _mod
            except ImportError:
                pass
        _mod = sys.modules["antenv.axon_hooks"]
        if _mod.get_axon_ntff_profile_hook() is None:
            from trn_agent_boot.trn_boot import _ntff_profile_via_ctypes
            _mod.set_axon_ntff_profile_hook(
                _ntff_profile_via_ctypes("/opt/axon/libaxon_pjrt.so"))
        import concourse.bass_utils as _bu
        _bu.upload_artifacts = lambda d: d  # no cloud copy in this container
        kwargs = dict(trace=True, trace_cores=[0])
        if _trace_dir is not None:
            kwargs["tmpdir"] = _trace_dir
    res = run_bass_kernel_spmd(nc, in_maps, core_ids=list(range(NCORES)),
                               **kwargs)

    out = np.empty((B, S, D), dtype=np.float32)
    bo32 = np.asarray(bo, np.float32)
    for b in range(B):
        acc = _unarrange_out(np.asarray(res.results[b * NGROUPS]["out_a"]))
        for g in range(1, NGROUPS):
            acc = acc + _unarrange_out(
                np.asarray(res.results[b * NGROUPS + g]["out_a"]))
        out[b] = acc + bo32
    if _profile:
        kernel._last_exec_time_ns = res.exec_time_ns
        kernel._last_results = res
    return out
